# revision 33
# baseline (speedup 1.0000x reference)
"""DiffMHA (differential multi-head attention) block on 8 TRN2 NeuronCores.

Problem: B=4, L=1024, D=1024, H=16 heads (DH=64). Three input streams
(e_v, e_a0, e_a1); Q/K projections per stream, scores summed across
streams, causal-masked softmax, context from the v-stream values,
out-projection + residual + LayerNorm.

Sharding: (batch, head-half) -> 8 cores. Core c handles batch c//2 and
heads (c%2)*8 .. (c%2)*8+8. Each core computes its 8 heads' Q/K/V
projections (512 of 1024 channels), scores + softmax + context. Fold
context halves (128 channels x 512 rows) are exchanged between the two
cores of a batch via per-fold pairwise AllToAll DURING attention; each
core then runs the out-projection for its own 512 sequence rows with
the full 1024-channel contraction, then residual + LayerNorm locally.
No end-of-kernel collective.

Key optimizations over the v1 kernel:
- Causal skipping: score tiles with kt*128 > qb*256+255 are fully
  masked and skipped entirely (exp == 0 exactly); only diagonal-band
  tiles get the mask add. Attention matmul work drops ~40%.
- Stream packing: q/k of streams v and a0 are packed onto 128
  partitions (64 chans each) at projection-eviction time (partition-
  shifted PSUM->SBUF copies), so their two 64-deep score matmuls fuse
  into one 128-deep matmul; stream a1 stays a 64-deep accumulate.
- PSUM bank alternation: consecutive PE matmuls never accumulate into
  the same PSUM bank back-to-back (V-proj lf pairs, per-head score
  tiles, ctx of 2 heads, out-proj chains are interleaved), which keeps
  the PE at its ~216ns/512-col pipelined rate instead of ~430ns.
- Softmax normalization deferred past ctx accumulation via the extra
  ones-row of V (unchanged), reciprocal+broadcast per (head, q-half).
- DMA order: xt_v + wv first so the PE starts ~10us in, not ~46us.
"""

import os
import sys
import types

import ml_dtypes
import numpy as np

B, L, D, H = 4, 1024, 1024, 16
DH = D // H
HPC = H // 2  # heads per core
C = HPC * DH  # channels per core (512)
SCALE = float(1.0 / np.sqrt(DH))
EPS = 1e-12
NCORES = 8
BF16 = ml_dtypes.bfloat16


def _install_ntff_hook():
    """Recreate antenv.axon_hooks (absent in this image) so
    run_bass_kernel_spmd(trace=True) can capture NTFF profiles."""
    if "antenv.axon_hooks" in sys.modules:
        return
    try:
        from trn_agent_boot.trn_boot import _ntff_profile_via_ctypes

        hook = _ntff_profile_via_ctypes("/opt/axon/libaxon_pjrt.so")
    except Exception:
        hook = None
    mod = types.ModuleType("antenv.axon_hooks")
    mod.get_axon_ntff_profile_hook = lambda: hook
    mod.set_axon_ntff_profile_hook = lambda h: None
    sys.modules["antenv.axon_hooks"] = mod


_install_ntff_hook()

import concourse.bass as bass  # noqa: E402
import concourse.mybir as mybir  # noqa: E402
import concourse.tile as tile  # noqa: E402
from concourse import bacc  # noqa: E402
from concourse.bass_utils import run_bass_kernel_spmd  # noqa: E402

F32 = mybir.dt.float32
BF = mybir.dt.bfloat16
AF = mybir.ActivationFunctionType
ALU = mybir.AluOpType

_NC_CACHE = {}
LAST_RESULT = None

NQF = C // 128  # 4 channel folds per stream (2 heads each)
NLT = L // 128  # 8 l-tiles
NDT = D // 128  # 8 d-tiles (contraction)
NKT = L // 128  # 8 k-tiles
NRF = (L // 2) // 128  # 4 row tiles for out-proj/LN
STREAMS = ("v", "a0", "a1")
GROUPS = [[0, 1], [2, 3], [4, 5], [6, 7]]


def build_nc():
    nc = bacc.Bacc("TRN2", target_bir_lowering=False, debug=False, num_devices=NCORES)

    # ---- DRAM parameters (per-core shards, host-prepped) ----
    xt = {s: nc.declare_dram_parameter(f"xt_{s}", [D, L], BF, isOutput=False) for s in STREAMS}
    wq = {s: nc.declare_dram_parameter(f"wq_{s}", [NQF, D, 128], BF, isOutput=False) for s in STREAMS}
    wk = {s: nc.declare_dram_parameter(f"wk_{s}", [NQF, D, 128], BF, isOutput=False) for s in STREAMS}
    wv = nc.declare_dram_parameter("wv", [D, C], BF, isOutput=False)
    wout8 = nc.declare_dram_parameter("wout8", [128, 8, D], BF, isOutput=False)
    bq = {s: nc.declare_dram_parameter(f"bq_{s}", [C], F32, isOutput=False) for s in STREAMS}
    bk = {s: nc.declare_dram_parameter(f"bk_{s}", [C], F32, isOutput=False) for s in STREAMS}
    bv = nc.declare_dram_parameter("bv", [1, C], BF, isOutput=False)
    maskAB = nc.declare_dram_parameter("maskAB", [128, 2, 256], BF, isOutput=False)
    sel = nc.declare_dram_parameter("sel", [128, 2, 512], BF, isOutput=False)
    ev_res = nc.declare_dram_parameter("ev_res", [L // 2, D], BF, isOutput=False)
    eye = nc.declare_dram_parameter("eye", [128, 128], BF, isOutput=False)
    gamma = nc.declare_dram_parameter("gamma", [1, D], F32, isOutput=False)
    beta = nc.declare_dram_parameter("beta", [1, D], F32, isOutput=False)
    out = nc.declare_dram_parameter("out", [L // 2, D], F32, isOutput=True)


    with tile.TileContext(nc) as tc:
        with (
            tc.tile_pool(name="persist", bufs=1) as persist,
            tc.tile_pool(name="xtp", bufs=1) as xtp,
            tc.tile_pool(name="wf", bufs=10) as wf,
            tc.tile_pool(name="qkf", bufs=2) as qkf,
            tc.tile_pool(name="small", bufs=4) as small,
            tc.tile_pool(name="attn", bufs=4) as attn_pool,
            tc.tile_pool(name="ln", bufs=4) as ln_pool,
            tc.tile_pool(name="evp", bufs=4) as evp,
            tc.tile_pool(name="ctxf", bufs=2) as ctxf_pool,
            tc.tile_pool(name="proj_ps", bufs=3, space="PSUM") as proj_ps,
            tc.tile_pool(name="sc_ps", bufs=3, space="PSUM") as sc_ps,
            tc.tile_pool(name="ctx_ps", bufs=2, space="PSUM") as ctx_ps,
            tc.tile_pool(name="dram", bufs=1, space="DRAM") as dram,
        ):
            # ---- persistent SBUF tensors ----
            vnat = persist.tile([128, NLT, HPC, DH + 1], BF, tag="vnat")
            ctx_all = persist.tile([128, 8, L // 2], BF, tag="ctxall")
            mask_sb = persist.tile([128, 2, 256], BF, tag="maskAB")
            ones_b = persist.tile([1, L], BF, tag="ones")
            gb_bc = persist.tile([128, 2, D], F32, tag="gbbc")
            bv_sb = persist.tile([1, C], BF, tag="bvsb")
            wv_sb = persist.tile([128, NDT, C], BF, tag="wvsb")
            wout_sb = persist.tile([128, 8, D], BF, tag="woutsb")
            eps_sb = persist.tile([128, 1], F32, tag="eps")
            eye_sb = persist.tile([128, 128], BF, tag="eye")
            bq_sb = {
                s: persist.tile([128, NQF], F32, tag=f"bq{s}", name=f"bq_sb_{s}")
                for s in STREAMS
            }
            bk_sb = {
                s: persist.tile([128, NQF], F32, tag=f"bk{s}", name=f"bk_sb_{s}")
                for s in STREAMS
            }

            # ---- fold weight loader (lazy, cached) ----
            wf_cache = {}

            def load_wf(s, ff):
                if (s, ff) in wf_cache:
                    return wf_cache[(s, ff)]
                wq_t = wf.tile([128, NDT, 128], BF, tag="w", name=f"wq_{s}{ff}")
                wk_t = wf.tile([128, NDT, 128], BF, tag="w", name=f"wk_{s}{ff}")
                nc.sync.dma_start(
                    out=wq_t[:, :, :],
                    in_=wq[s][ff, :, :].rearrange("(dt p) c -> p dt c", p=128),
                )
                nc.sync.dma_start(
                    out=wk_t[:, :, :],
                    in_=wk[s][ff, :, :].rearrange("(dt p) c -> p dt c", p=128),
                )
                wf_cache[(s, ff)] = (wq_t, wk_t)
                return wq_t, wk_t

            # ---- preamble DMAs, in critical-path order: V-proj deps
            #      first, then fold-0 Q/K weights interleaved with the
            #      remaining embeddings; big late-use tensors (wout, ev,
            #      gamma/beta) are deferred into the fold loop. ----
            xt_sb = {}
            t = xtp.tile([128, NDT, L], BF, tag="xtv", name="xt_sb_v")
            nc.sync.dma_start(
                out=t[:, :, :], in_=xt["v"][:, :].rearrange("(dt p) l -> p dt l", p=128)
            )
            xt_sb["v"] = t
            load_wf("v", 0)
            for s in ("a0", "a1"):
                t = xtp.tile([128, NDT, L], BF, tag=f"xt{s}", name=f"xt_sb_{s}")
                nc.sync.dma_start(
                    out=t[:, :, :],
                    in_=xt[s][:, :].rearrange("(dt p) l -> p dt l", p=128),
                )
                xt_sb[s] = t
                load_wf(s, 0)
            nc.sync.dma_start(
                out=wv_sb[:, :, :], in_=wv[:, :].rearrange("(dt p) c -> p dt c", p=128)
            )
            nc.sync.dma_start(out=bv_sb[:, :], in_=bv[:, :])

            nc.vector.memset(ones_b[:, :], 1.0)
            nc.vector.memset(eps_sb[:, :], EPS)
            nc.vector.memset(vnat[:, :, :, DH : DH + 1], 1.0)

            nc.sync.dma_start(out=mask_sb[:, :, :], in_=maskAB[:, :, :])
            sel_sb = persist.tile([128, 2, 512], BF, tag="sel")
            nc.sync.dma_start(out=sel_sb[:, :, :], in_=sel[:, :, :])
            nc.sync.dma_start(out=eye_sb[:, :], in_=eye[:, :])
            for s in STREAMS:
                nc.sync.dma_start(
                    out=bq_sb[s][:, :], in_=bq[s][:].rearrange("(f p) -> p f", p=128)
                )
                nc.sync.dma_start(
                    out=bk_sb[s][:, :], in_=bk[s][:].rearrange("(f p) -> p f", p=128)
                )
            ev_sb = []

            def emit_vproj():
                # ---- V projection: natural [l, c] layout + ones column.
                #      lf pairs interleaved so consecutive matmuls alternate
                #      PSUM banks. ----
                for pair in range(NLT // 2):
                    lf0, lf1 = 2 * pair, 2 * pair + 1
                    psA = sc_ps.tile([128, C], F32, tag="sc")
                    psB = sc_ps.tile([128, C], F32, tag="sc")
                    for dt in range(NDT):
                        for lf, ps in ((lf0, psA), (lf1, psB)):
                            nc.tensor.matmul(
                                ps[:, :],
                                xt_sb["v"][:, dt, lf * 128 : (lf + 1) * 128],
                                wv_sb[:, dt, :],
                                start=(dt == 0),
                                stop=False,
                            )
                    for lf, ps in ((lf0, psA), (lf1, psB)):
                        nc.tensor.matmul(
                            ps[:, :],
                            ones_b[:, lf * 128 : (lf + 1) * 128],
                            bv_sb[:, :],
                            start=False,
                            stop=True,
                        )
                    nc.scalar.copy(vnat[:, lf0, :, 0:DH], psA[:, :])
                    nc.scalar.copy(vnat[:, lf1, :, 0:DH], psB[:, :])

            # ---- fold-major main loop. Fold f+1's projections are
            #      emitted BEFORE fold f's attention so the packed-Q/K
            #      eviction latency hides under attention compute. ----
            fold_tiles = {}

            def emit_proj(f):
                # packed tiles: partitions [0:64] = stream v chans of the
                # head, [64:128] = stream a0 chans; a1 keeps fold layout.
                qpk = [
                    qkf.tile([128, L], BF, tag=f"qpk{hh}", name=f"qpk{hh}_{f}")
                    for hh in range(2)
                ]
                kpk = [
                    qkf.tile([128, L], BF, tag=f"kpk{hh}", name=f"kpk{hh}_{f}")
                    for hh in range(2)
                ]
                qa1 = qkf.tile([128, L], BF, tag="qa1", name=f"qa1_{f}")
                ka1 = qkf.tile([128, L], BF, tag="ka1", name=f"ka1_{f}")
                for s in STREAMS:
                    wq_t, wk_t = load_wf(s, f)
                    for w_t, b_t, pk, a1t in (
                        (wq_t, bq_sb[s], qpk, qa1),
                        (wk_t, bk_sb[s], kpk, ka1),
                    ):
                        ps = [
                            proj_ps.tile([128, 512], F32, tag="proj", name=f"pp{lh}")
                            for lh in range(2)
                        ]
                        for dt in range(NDT):
                            for lh in range(2):
                                nc.tensor.matmul(
                                    ps[lh][:, :],
                                    w_t[:, dt, :],
                                    xt_sb[s][:, dt, lh * 512 : (lh + 1) * 512],
                                    start=(dt == 0),
                                    stop=(dt == NDT - 1),
                                )
                        for lh in range(2):
                            lsl = slice(lh * 512, (lh + 1) * 512)
                            if s == "a1":
                                nc.scalar.activation(
                                    a1t[:, lsl], ps[lh][:, :], AF.Identity,
                                    bias=b_t[:, f : f + 1],
                                )
                            else:
                                off = 0 if s == "v" else 64
                                for hh in range(2):
                                    nc.scalar.activation(
                                        pk[hh][off : off + 64, lsl],
                                        ps[lh][hh * 64 : hh * 64 + 64, :],
                                        AF.Identity,
                                        bias=b_t[hh * 64 : hh * 64 + 64, f : f + 1],
                                    )
                fold_tiles[f] = (qpk, kpk, qa1, ka1)

            def emit_attention(f):
                qpk, kpk, qa1, ka1 = fold_tiles.pop(f)

                # stage late-use loads here so they don't compete with the
                # critical-path preamble/projection DMAs
                if f == 0:
                    nc.sync.dma_start(out=wout_sb[:, :, :], in_=wout8[:, :, :])
                if f == 1:
                    for rf in range(NRF):
                        t = evp.tile([128, D], BF, tag="ev", name=f"ev{rf}")
                        nc.sync.dma_start(
                            out=t[:, :], in_=ev_res[rf * 128 : (rf + 1) * 128, :]
                        )
                        ev_sb.append(t)
                if f == 2:
                    gsb = small.tile([1, D], F32, tag="gsb", bufs=1)
                    bsb = small.tile([1, D], F32, tag="bsb", bufs=1)
                    nc.sync.dma_start(out=gsb[:, :], in_=gamma[:, :])
                    nc.sync.dma_start(out=bsb[:, :], in_=beta[:, :])
                    nc.gpsimd.partition_broadcast(gb_bc[:, 0, :], gsb[:, :])
                    nc.gpsimd.partition_broadcast(gb_bc[:, 1, :], bsb[:, :])

                ctxf = ctxf_pool.tile([128, L], BF, tag="ctxf", name=f"ctxf{f}")
                cxs = ctxf_pool.tile(
                    [128, 2, 2, 512], BF, tag="cxs", name=f"cxs{f}", bufs=1
                )
                cx_in = dram.tile(
                    [2, 2, 128, 512], BF, tag=f"cxin{f}", name=f"cxin{f}"
                )
                for qh in range(2):
                    cps = [
                        ctx_ps.tile([DH + 1, 512], F32, tag="ctx", name=f"cps{i}")
                        for i in range(2)
                    ]
                    n_kt = 4 * qh + 4  # live k-tiles for this q-half
                    sps_at = {}

                    def emit_scores(kt):
                        sps = [
                            sc_ps.tile([128, 512], F32, tag="sc", name=f"sps{i}")
                            for i in range(2)
                        ]
                        # PE issue is ~216ns/instr regardless of width, so
                        # use one full 512-col matmul pair when both q
                        # halves are live; 256-col only on the causal edge.
                        full = kt <= 4 * qh + 1
                        qsl = (
                            slice(qh * 512, qh * 512 + 512)
                            if full
                            else slice(qh * 512 + 256, qh * 512 + 512)
                        )
                        osl = slice(0, 512) if full else slice(256, 512)
                        ksl = slice(kt * 128, (kt + 1) * 128)
                        for hh in range(2):
                            nc.tensor.matmul(
                                sps[hh][:, osl],
                                kpk[hh][:, ksl],
                                qpk[hh][:, qsl],
                                start=True,
                                stop=False,
                            )
                        for hh in range(2):
                            p0 = hh * 64
                            nc.tensor.matmul(
                                sps[hh][:, osl],
                                ka1[p0 : p0 + 64, ksl],
                                qa1[p0 : p0 + 64, qsl],
                                start=False,
                                stop=True,
                            )
                        # mask only on diagonal-band halves
                        for qbh in range(2):
                            qb = 2 * qh + qbh
                            if kt in (2 * qb, 2 * qb + 1):
                                msl = slice(qbh * 256, qbh * 256 + 256)
                                for hh in range(2):
                                    nc.vector.tensor_add(
                                        sps[hh][:, msl],
                                        sps[hh][:, msl],
                                        mask_sb[:, kt % 2, :],
                                    )
                        # exp -> bf16 attn tiles (dead qb0 half zeroed so
                        # the full-width ctx matmul accumulates one group
                        # per PSUM bank)
                        at = [
                            attn_pool.tile([128, 512], BF, tag="attn", name=f"at{i}")
                            for i in range(2)
                        ]
                        for hh in range(2):
                            if not full:
                                nc.vector.memset(at[hh][:, 0:256], 0.0)
                            nc.scalar.activation(
                                at[hh][:, osl], sps[hh][:, osl], AF.Exp, scale=SCALE
                            )
                        sps_at[kt] = at

                    def emit_ctx(kt):
                        at = sps_at.pop(kt)
                        for hh in range(2):
                            h = 2 * f + hh
                            nc.tensor.matmul(
                                cps[hh][:, :],
                                vnat[:, kt, h, :],
                                at[hh][:, :],
                                start=(kt == 0),
                                stop=(kt == n_kt - 1),
                            )

                    prev = None
                    for kt in range(n_kt):
                        emit_scores(kt)
                        if prev is not None:
                            emit_ctx(prev)
                        prev = kt
                    emit_ctx(prev)

                    # normalize: divide ctx rows by the attn row-sums that
                    # accumulated in psum row DH (sum staged to SBUF for the
                    # fast custom-DVE reciprocal, which is SBUF-only)
                    for hh in range(2):
                        p0 = hh * 64
                        sr = small.tile([1, 512], F32, tag="sr", bufs=2)
                        nc.scalar.copy(sr[:, :], cps[hh][DH : DH + 1, :])
                        inv = small.tile([1, 512], F32, tag="inv", bufs=2)
                        nc.vector.reciprocal_approx_fast(inv[:, :], sr[:, :])
                        inv_bc = small.tile([64, 512], F32, tag="invbc", bufs=2)
                        nc.gpsimd.partition_broadcast(inv_bc[:, :], inv[:, :])
                        nc.vector.tensor_mul(
                            ctxf[p0 : p0 + 64, qh * 512 : (qh + 1) * 512],
                            cps[hh][0:DH, :],
                            inv_bc[:, :],
                        )

                    # stage this q-half (= dest-rank chunk) for the
                    # exchange as soon as it is normalized
                    for s2 in range(2):
                        nc.vector.tensor_mul(
                            cxs[:, qh, s2, :],
                            ctxf[:, qh * 512 : (qh + 1) * 512],
                            sel_sb[:, s2, :],
                        )
                        nc.sync.dma_start(
                            out=cx_in[qh, s2, :, :], in_=cxs[:, qh, s2, :]
                        )

                # -- exchange fold ctx halves with the pair core.
                # AllToAll isn't available for 2-core groups, so emulate it
                # with a ReduceScatter over [dest d][chan-slot s] staging
                # where slot s != own-half is zeroed via the host-provided
                # 0/1 `sel` tensor (x + 0 is exact in bf16). Rank d then
                # receives [ctx_half0, ctx_half1] for its own rows. --
                cx_out = dram.tile(
                    [2, 128, 512], BF, tag=f"cxout{f}", name=f"cxout{f}"
                )
                nc.gpsimd.collective_compute(
                    "ReduceScatter",
                    ALU.add,
                    replica_groups=GROUPS,
                    ins=[cx_in.opt()],
                    outs=[cx_out.opt()],
                )
                for s2 in range(2):
                    nc.sync.dma_start(
                        out=ctx_all[:, s2 * NQF + f, :], in_=cx_out[s2, :, :]
                    )

            emit_proj(0)
            for f in range(NQF):
                if f + 1 < NQF:
                    emit_proj(f + 1)
                if f == 0:
                    emit_vproj()
                emit_attention(f)

            # ---- out-projection over full 1024 channels for own rows ----
            # 8 chains (lt, dh). Chains for lt 0,1,3 are partially
            # accumulated (folds 0-2 contributions + residual) right after
            # fold-3 attention, filling the PE idle window while fold 3's
            # normalize/exchange runs; the fold-3 contributions and the lt2
            # chains run after the last readback.
            early_chains = [(lt, dh) for lt in (0, 1, 3) for dh in range(2)]
            late_chains = [(2, 0), (2, 1)]
            pools = [sc_ps, proj_ps]
            ptags = ["sc", "proj"]
            ops = {}
            for i, ch in enumerate(early_chains):
                ops[ch] = pools[i % 2].tile(
                    [128, 512], F32, tag=ptags[i % 2], name=f"opse{i}"
                )

            def op_mm(ch, cf, start, stop):
                lt, dh = ch
                nc.tensor.matmul(
                    ops[ch][:, :],
                    ctx_all[:, cf, lt * 128 : (lt + 1) * 128],
                    wout_sb[:, cf, dh * 512 : (dh + 1) * 512],
                    start=start,
                    stop=stop,
                )

            def op_eye(ch, stop):
                lt, dh = ch
                nc.tensor.matmul(
                    ops[ch][:, :],
                    eye_sb[:, :],
                    ev_sb[lt][:, dh * 512 : (dh + 1) * 512],
                    start=False,
                    stop=stop,
                )

            for cf in (0, 1, 2, 4, 5, 6):
                for ch in early_chains:
                    op_mm(ch, cf, start=(cf == 0), stop=False)
            for ch in early_chains:
                op_eye(ch, stop=False)
            # ---- late part: fold-3 contributions ----
            for cf in (3, 7):
                for ch in early_chains:
                    op_mm(ch, cf, start=False, stop=(cf == 7))
            for i, ch in enumerate(late_chains):
                ops[ch] = pools[i % 2].tile(
                    [128, 512], F32, tag=ptags[i % 2], name=f"opsl{i}"
                )
            for cf in range(8):
                for ch in late_chains:
                    op_mm(ch, cf, start=(cf == 0), stop=False)
            for ch in late_chains:
                op_eye(ch, stop=True)

            # ---- evict + LayerNorm per row tile ----
            for lt in (0, 1, 3, 2):
                lsl = slice(lt * 128, (lt + 1) * 128)
                xt_ = ln_pool.tile([128, D], F32, tag="x", name=f"x{lt}")
                nc.scalar.copy(xt_[:, 0:512], ops[(lt, 0)][:, :])
                nc.scalar.copy(xt_[:, 512:1024], ops[(lt, 1)][:, :])
                stats = small.tile([128, 2, 6], F32, tag="stats")
                nc.vector.bn_stats(out=stats[:, 0, :], in_=xt_[:, 0:512])
                nc.vector.bn_stats(out=stats[:, 1, :], in_=xt_[:, 512:1024])
                mv = small.tile([128, 2], F32, tag="mv")
                nc.vector.bn_aggr(out=mv[:, :], in_=stats[:, :, :])
                std = small.tile([128, 1], F32, tag="std")
                nc.scalar.activation(std[:, :], mv[:, 1:2], AF.Sqrt, bias=eps_sb[:, :])
                rstd = small.tile([128, 1], F32, tag="rstd")
                nc.vector.reciprocal(rstd[:, :], std[:, :])
                negmb = small.tile([128, 1], F32, tag="negmb")
                nc.vector.scalar_tensor_tensor(
                    negmb[:, :],
                    mv[:, 0:1],
                    -1.0,
                    rstd[:, :],
                    op0=ALU.mult,
                    op1=ALU.mult,
                )
                dacc = small.tile([128, 1], F32, tag="dacc")
                nc.vector.affine_mul_reduce(
                    xt_[:, :],
                    dacc[:, :],
                    xt_[:, :],
                    gb_bc[:, 0, :],
                    scale=rstd[:, :],
                    bias=negmb[:, :],
                )
                nc.vector.tensor_add(xt_[:, :], xt_[:, :], gb_bc[:, 1, :])
                nc.sync.dma_start(out=out[lsl, :], in_=xt_[:, :])

    nc.compile()
    return nc


def _get_nc():
    if "nc" not in _NC_CACHE:
        _NC_CACHE["nc"] = build_nc()
    return _NC_CACHE["nc"]


def kernel(
    e_v, e_a0, e_a1, Wqv, bqv, Wkv, bkv, Wvv, bvv,
    Wqa0, bqa0, Wka0, bka0, Wqa1, bqa1, Wka1, bka1,
    Wout, bout, ln_gamma, ln_beta, attn_mask,
):
    global LAST_RESULT
    f = np.asarray
    e_v, e_a0, e_a1 = f(e_v), f(e_a0), f(e_a1)
    attn_mask = f(attn_mask)
    c32 = lambda a: np.ascontiguousarray(a, dtype=np.float32)
    cbf = lambda a: np.ascontiguousarray(np.asarray(a, dtype=np.float32).astype(BF16))

    wq_full = {"v": f(Wqv), "a0": f(Wqa0), "a1": f(Wqa1)}
    wk_full = {"v": f(Wkv), "a0": f(Wka0), "a1": f(Wka1)}
    bq_full = {"v": f(bqv), "a0": f(bqa0), "a1": f(bqa1)}
    bk_full = {"v": f(bkv), "a0": f(bka0), "a1": f(bka1)}

    xts = {}
    maskABs = {}
    for b in range(B):
        xts[b] = {
            "v": cbf(e_v[b].T),
            "a0": cbf(e_a0[b].T),
            "a1": cbf(e_a1[b].T),
        }
        mT = f(attn_mask[b, 0]).T * (1.0 / SCALE)
        # diagonal-band mask patterns: A = (kt == 2*qb), B = (kt == 2*qb+1)
        maskABs[b] = cbf(np.stack([mT[0:128, 0:256], mT[128:256, 0:256]], axis=1))

    def fold_slice(w, S):
        # [D, C] slice -> [NQF, D, 128] fold-major
        ws = np.asarray(w[:, S], dtype=np.float32)  # [D, C]
        return np.ascontiguousarray(
            ws.reshape(D, NQF, 128).transpose(1, 0, 2).astype(BF16)
        )

    Wout_f = f(Wout).astype(np.float32)
    bout_f = f(bout).astype(np.float32)
    # [128 p, 8 cf, D] with cf = s*4+f mapping Wout rows s*512+f*128+p
    wout8 = np.ascontiguousarray(
        Wout_f.reshape(2, NQF, 128, D).transpose(2, 0, 1, 3).astype(BF16)
    ).reshape(128, 8, D)

    in_maps = []
    for c in range(NCORES):
        b, hh = c // 2, c % 2
        S = slice(hh * C, (hh + 1) * C)
        m = {}
        for s in STREAMS:
            m[f"xt_{s}"] = xts[b][s]
            m[f"wq_{s}"] = fold_slice(wq_full[s], S)
            m[f"wk_{s}"] = fold_slice(wk_full[s], S)
            m[f"bq_{s}"] = c32(bq_full[s][S])
            m[f"bk_{s}"] = c32(bk_full[s][S])
        m["wv"] = cbf(f(Wvv)[:, S])
        m["bv"] = cbf(f(bvv)[S]).reshape(1, C)
        m["wout8"] = wout8
        m["maskAB"] = maskABs[b]
        selv = np.zeros((128, 2, 512), dtype=np.float32)
        selv[:, hh, :] = 1.0
        m["sel"] = cbf(selv)
        m["ev_res"] = cbf(e_v[b, hh * 512 : (hh + 1) * 512, :] + bout_f[None, :])
        m["eye"] = cbf(np.eye(128, dtype=np.float32))
        m["gamma"] = c32(f(ln_gamma)).reshape(1, D)
        m["beta"] = c32(f(ln_beta)).reshape(1, D)
        in_maps.append(m)

    nc = _get_nc()
    trace = bool(os.environ.get("KERNEL_TRACE"))
    res = run_bass_kernel_spmd(
        nc, in_maps, core_ids=list(range(NCORES)), trace=trace
    )
    LAST_RESULT = res

    out_full = np.empty((B, L, D), dtype=np.float32)
    for c in range(NCORES):
        b, hh = c // 2, c % 2
        out_full[b, hh * 512 : (hh + 1) * 512, :] = res.results[c]["out"]
    return out_full


# revision 34
# speedup vs baseline: 1.0571x; 1.0571x over previous
"""DiffMHA (differential multi-head attention) block on 8 TRN2 NeuronCores.

Problem: B=4, L=1024, D=1024, H=16 heads (DH=64). Three input streams
(e_v, e_a0, e_a1); Q/K projections per stream, scores summed across
streams, causal-masked softmax, context from the v-stream values,
out-projection + residual + LayerNorm.

Sharding: (batch, head-half) -> 8 cores. Core c handles batch c//2 and
heads (c%2)*8 .. (c%2)*8+8. Each core computes its 8 heads' Q/K/V
projections (512 of 1024 channels), scores + softmax + context. Fold
context halves (128 channels x 512 rows) are exchanged between the two
cores of a batch via per-fold pairwise AllToAll DURING attention; each
core then runs the out-projection for its own 512 sequence rows with
the full 1024-channel contraction, then residual + LayerNorm locally.
No end-of-kernel collective.

Key optimizations over the v1 kernel:
- Causal skipping: score tiles with kt*128 > qb*256+255 are fully
  masked and skipped entirely (exp == 0 exactly); only diagonal-band
  tiles get the mask add. Attention matmul work drops ~40%.
- Stream packing: q/k of streams v and a0 are packed onto 128
  partitions (64 chans each) at projection-eviction time (partition-
  shifted PSUM->SBUF copies), so their two 64-deep score matmuls fuse
  into one 128-deep matmul; stream a1 stays a 64-deep accumulate.
- PSUM bank alternation: consecutive PE matmuls never accumulate into
  the same PSUM bank back-to-back (V-proj lf pairs, per-head score
  tiles, ctx of 2 heads, out-proj chains are interleaved), which keeps
  the PE at its ~216ns/512-col pipelined rate instead of ~430ns.
- Softmax normalization deferred past ctx accumulation via the extra
  ones-row of V (unchanged), reciprocal+broadcast per (head, q-half).
- DMA order: xt_v + wv first so the PE starts ~10us in, not ~46us.
"""

import os
import sys
import types

import ml_dtypes
import numpy as np

B, L, D, H = 4, 1024, 1024, 16
DH = D // H
HPC = H // 2  # heads per core
C = HPC * DH  # channels per core (512)
SCALE = float(1.0 / np.sqrt(DH))
EPS = 1e-12
NCORES = 8
BF16 = ml_dtypes.bfloat16


def _install_ntff_hook():
    """Recreate antenv.axon_hooks (absent in this image) so
    run_bass_kernel_spmd(trace=True) can capture NTFF profiles."""
    if "antenv.axon_hooks" in sys.modules:
        return
    try:
        from trn_agent_boot.trn_boot import _ntff_profile_via_ctypes

        hook = _ntff_profile_via_ctypes("/opt/axon/libaxon_pjrt.so")
    except Exception:
        hook = None
    mod = types.ModuleType("antenv.axon_hooks")
    mod.get_axon_ntff_profile_hook = lambda: hook
    mod.set_axon_ntff_profile_hook = lambda h: None
    sys.modules["antenv.axon_hooks"] = mod


_install_ntff_hook()

import concourse.bass as bass  # noqa: E402
import concourse.mybir as mybir  # noqa: E402
import concourse.tile as tile  # noqa: E402
from concourse import bacc  # noqa: E402
from concourse.bass_utils import run_bass_kernel_spmd  # noqa: E402

F32 = mybir.dt.float32
BF = mybir.dt.bfloat16
AF = mybir.ActivationFunctionType
ALU = mybir.AluOpType

_NC_CACHE = {}
LAST_RESULT = None

NQF = C // 128  # 4 channel folds per stream (2 heads each)
NLT = L // 128  # 8 l-tiles
NDT = D // 128  # 8 d-tiles (contraction)
NKT = L // 128  # 8 k-tiles
NRF = (L // 2) // 128  # 4 row tiles for out-proj/LN
STREAMS = ("v", "a0", "a1")
GROUPS = [[0, 1], [2, 3], [4, 5], [6, 7]]


def build_nc():
    nc = bacc.Bacc("TRN2", target_bir_lowering=False, debug=False, num_devices=NCORES)

    # ---- DRAM parameters (per-core shards, host-prepped) ----
    xt = {s: nc.declare_dram_parameter(f"xt_{s}", [D, L], BF, isOutput=False) for s in STREAMS}
    wq = {s: nc.declare_dram_parameter(f"wq_{s}", [NQF, D, 128], BF, isOutput=False) for s in STREAMS}
    wk = {s: nc.declare_dram_parameter(f"wk_{s}", [NQF, D, 128], BF, isOutput=False) for s in STREAMS}
    wv = nc.declare_dram_parameter("wv", [D, C], BF, isOutput=False)
    wout8 = nc.declare_dram_parameter("wout8", [128, 8, D], BF, isOutput=False)
    bq = {s: nc.declare_dram_parameter(f"bq_{s}", [C], F32, isOutput=False) for s in STREAMS}
    bk = {s: nc.declare_dram_parameter(f"bk_{s}", [C], F32, isOutput=False) for s in STREAMS}
    bv = nc.declare_dram_parameter("bv", [1, C], BF, isOutput=False)
    maskAB = nc.declare_dram_parameter("maskAB", [128, 2, 256], BF, isOutput=False)
    sel = nc.declare_dram_parameter("sel", [128, 2, 512], BF, isOutput=False)
    ev_res = nc.declare_dram_parameter("ev_res", [L // 2, D], BF, isOutput=False)
    eye = nc.declare_dram_parameter("eye", [128, 128], BF, isOutput=False)
    gamma = nc.declare_dram_parameter("gamma", [1, D], F32, isOutput=False)
    beta = nc.declare_dram_parameter("beta", [1, D], F32, isOutput=False)
    out = nc.declare_dram_parameter("out", [L // 2, D], F32, isOutput=True)


    with tile.TileContext(nc) as tc:
        with (
            tc.tile_pool(name="persist", bufs=1) as persist,
            tc.tile_pool(name="xtp", bufs=1) as xtp,
            tc.tile_pool(name="wf", bufs=10) as wf,
            tc.tile_pool(name="qkf", bufs=2) as qkf,
            tc.tile_pool(name="small", bufs=4) as small,
            tc.tile_pool(name="attn", bufs=4) as attn_pool,
            tc.tile_pool(name="ln", bufs=4) as ln_pool,
            tc.tile_pool(name="evp", bufs=4) as evp,
            tc.tile_pool(name="ctxf", bufs=2) as ctxf_pool,
            tc.tile_pool(name="proj_ps", bufs=3, space="PSUM") as proj_ps,
            tc.tile_pool(name="sc_ps", bufs=3, space="PSUM") as sc_ps,
            tc.tile_pool(name="ctx_ps", bufs=2, space="PSUM") as ctx_ps,
            tc.tile_pool(name="dram", bufs=1, space="DRAM") as dram,
        ):
            # ---- persistent SBUF tensors ----
            vnat = persist.tile([128, NLT, HPC, DH + 1], BF, tag="vnat")
            ctx_all = persist.tile([128, 8, L // 2], BF, tag="ctxall")
            mask_sb = persist.tile([128, 2, 256], BF, tag="maskAB")
            ones_b = persist.tile([1, L], BF, tag="ones")
            gb_bc = persist.tile([128, 2, D], F32, tag="gbbc")
            bv_sb = persist.tile([1, C], BF, tag="bvsb")
            wv_sb = persist.tile([128, NDT, C], BF, tag="wvsb")
            wout_sb = persist.tile([128, 8, D], BF, tag="woutsb")
            eps_sb = persist.tile([128, 1], F32, tag="eps")
            eye_sb = persist.tile([128, 128], BF, tag="eye")
            bq_sb = {
                s: persist.tile([128, NQF], F32, tag=f"bq{s}", name=f"bq_sb_{s}")
                for s in STREAMS
            }
            bk_sb = {
                s: persist.tile([128, NQF], F32, tag=f"bk{s}", name=f"bk_sb_{s}")
                for s in STREAMS
            }

            # ---- fold weight loader (lazy, cached) ----
            wf_cache = {}

            def load_wf(s, ff):
                if (s, ff) in wf_cache:
                    return wf_cache[(s, ff)]
                wq_t = wf.tile([128, NDT, 128], BF, tag="w", name=f"wq_{s}{ff}")
                wk_t = wf.tile([128, NDT, 128], BF, tag="w", name=f"wk_{s}{ff}")
                nc.sync.dma_start(
                    out=wq_t[:, :, :],
                    in_=wq[s][ff, :, :].rearrange("(dt p) c -> p dt c", p=128),
                )
                nc.sync.dma_start(
                    out=wk_t[:, :, :],
                    in_=wk[s][ff, :, :].rearrange("(dt p) c -> p dt c", p=128),
                )
                wf_cache[(s, ff)] = (wq_t, wk_t)
                return wq_t, wk_t

            # ---- preamble DMAs, in critical-path order: V-proj deps
            #      first, then fold-0 Q/K weights interleaved with the
            #      remaining embeddings; big late-use tensors (wout, ev,
            #      gamma/beta) are deferred into the fold loop. ----
            xt_sb = {}
            t = xtp.tile([128, NDT, L], BF, tag="xtv", name="xt_sb_v")
            nc.sync.dma_start(
                out=t[:, :, :], in_=xt["v"][:, :].rearrange("(dt p) l -> p dt l", p=128)
            )
            xt_sb["v"] = t
            nc.sync.dma_start(
                out=wv_sb[:, :, :], in_=wv[:, :].rearrange("(dt p) c -> p dt c", p=128)
            )
            nc.sync.dma_start(out=bv_sb[:, :], in_=bv[:, :])
            load_wf("v", 0)
            for s in ("a0", "a1"):
                t = xtp.tile([128, NDT, L], BF, tag=f"xt{s}", name=f"xt_sb_{s}")
                nc.sync.dma_start(
                    out=t[:, :, :],
                    in_=xt[s][:, :].rearrange("(dt p) l -> p dt l", p=128),
                )
                xt_sb[s] = t
                load_wf(s, 0)

            nc.vector.memset(ones_b[:, :], 1.0)
            nc.vector.memset(eps_sb[:, :], EPS)
            nc.vector.memset(vnat[:, :, :, DH : DH + 1], 1.0)

            nc.sync.dma_start(out=mask_sb[:, :, :], in_=maskAB[:, :, :])
            sel_sb = persist.tile([128, 2, 512], BF, tag="sel")
            nc.sync.dma_start(out=sel_sb[:, :, :], in_=sel[:, :, :])
            nc.sync.dma_start(out=eye_sb[:, :], in_=eye[:, :])
            for s in STREAMS:
                nc.sync.dma_start(
                    out=bq_sb[s][:, :], in_=bq[s][:].rearrange("(f p) -> p f", p=128)
                )
                nc.sync.dma_start(
                    out=bk_sb[s][:, :], in_=bk[s][:].rearrange("(f p) -> p f", p=128)
                )
            ev_sb = []

            def emit_vproj():
                # ---- V projection: natural [l, c] layout + ones column.
                #      lf pairs interleaved so consecutive matmuls alternate
                #      PSUM banks. ----
                for pair in range(NLT // 2):
                    lf0, lf1 = 2 * pair, 2 * pair + 1
                    psA = sc_ps.tile([128, C], F32, tag="sc")
                    psB = sc_ps.tile([128, C], F32, tag="sc")
                    for dt in range(NDT):
                        for lf, ps in ((lf0, psA), (lf1, psB)):
                            nc.tensor.matmul(
                                ps[:, :],
                                xt_sb["v"][:, dt, lf * 128 : (lf + 1) * 128],
                                wv_sb[:, dt, :],
                                start=(dt == 0),
                                stop=False,
                            )
                    for lf, ps in ((lf0, psA), (lf1, psB)):
                        nc.tensor.matmul(
                            ps[:, :],
                            ones_b[:, lf * 128 : (lf + 1) * 128],
                            bv_sb[:, :],
                            start=False,
                            stop=True,
                        )
                    nc.scalar.copy(vnat[:, lf0, :, 0:DH], psA[:, :])
                    nc.scalar.copy(vnat[:, lf1, :, 0:DH], psB[:, :])

            # ---- fold-major main loop. Fold f+1's projections are
            #      emitted BEFORE fold f's attention so the packed-Q/K
            #      eviction latency hides under attention compute. ----
            fold_tiles = {}

            def emit_proj(f):
                # packed tiles: partitions [0:64] = stream v chans of the
                # head, [64:128] = stream a0 chans; a1 keeps fold layout.
                qpk = [
                    qkf.tile([128, L], BF, tag=f"qpk{hh}", name=f"qpk{hh}_{f}")
                    for hh in range(2)
                ]
                kpk = [
                    qkf.tile([128, L], BF, tag=f"kpk{hh}", name=f"kpk{hh}_{f}")
                    for hh in range(2)
                ]
                qa1 = qkf.tile([128, L], BF, tag="qa1", name=f"qa1_{f}")
                ka1 = qkf.tile([128, L], BF, tag="ka1", name=f"ka1_{f}")
                for s in STREAMS:
                    wq_t, wk_t = load_wf(s, f)
                    for w_t, b_t, pk, a1t in (
                        (wq_t, bq_sb[s], qpk, qa1),
                        (wk_t, bk_sb[s], kpk, ka1),
                    ):
                        ps = [
                            proj_ps.tile([128, 512], F32, tag="proj", name=f"pp{lh}")
                            for lh in range(2)
                        ]
                        for dt in range(NDT):
                            for lh in range(2):
                                nc.tensor.matmul(
                                    ps[lh][:, :],
                                    w_t[:, dt, :],
                                    xt_sb[s][:, dt, lh * 512 : (lh + 1) * 512],
                                    start=(dt == 0),
                                    stop=(dt == NDT - 1),
                                )
                        for lh in range(2):
                            lsl = slice(lh * 512, (lh + 1) * 512)
                            if s == "a1":
                                nc.scalar.activation(
                                    a1t[:, lsl], ps[lh][:, :], AF.Identity,
                                    bias=b_t[:, f : f + 1],
                                )
                            else:
                                off = 0 if s == "v" else 64
                                for hh in range(2):
                                    nc.scalar.activation(
                                        pk[hh][off : off + 64, lsl],
                                        ps[lh][hh * 64 : hh * 64 + 64, :],
                                        AF.Identity,
                                        bias=b_t[hh * 64 : hh * 64 + 64, f : f + 1],
                                    )
                fold_tiles[f] = (qpk, kpk, qa1, ka1)

            def emit_attention(f):
                qpk, kpk, qa1, ka1 = fold_tiles.pop(f)

                # stage late-use loads here so they don't compete with the
                # critical-path preamble/projection DMAs
                if f == 0:
                    nc.sync.dma_start(out=wout_sb[:, :, :], in_=wout8[:, :, :])
                if f == 1:
                    for rf in range(NRF):
                        t = evp.tile([128, D], BF, tag="ev", name=f"ev{rf}")
                        nc.sync.dma_start(
                            out=t[:, :], in_=ev_res[rf * 128 : (rf + 1) * 128, :]
                        )
                        ev_sb.append(t)
                if f == 2:
                    gsb = small.tile([1, D], F32, tag="gsb", bufs=1)
                    bsb = small.tile([1, D], F32, tag="bsb", bufs=1)
                    nc.sync.dma_start(out=gsb[:, :], in_=gamma[:, :])
                    nc.sync.dma_start(out=bsb[:, :], in_=beta[:, :])
                    nc.gpsimd.partition_broadcast(gb_bc[:, 0, :], gsb[:, :])
                    nc.gpsimd.partition_broadcast(gb_bc[:, 1, :], bsb[:, :])

                ctxf = ctxf_pool.tile([128, L], BF, tag="ctxf", name=f"ctxf{f}")
                cxs = ctxf_pool.tile(
                    [128, 2, 2, 512], BF, tag="cxs", name=f"cxs{f}", bufs=1
                )
                cx_in = dram.tile(
                    [2, 2, 128, 512], BF, tag=f"cxin{f}", name=f"cxin{f}"
                )
                for qh in range(2):
                    cps = [
                        ctx_ps.tile([DH + 1, 512], F32, tag="ctx", name=f"cps{i}")
                        for i in range(2)
                    ]
                    n_kt = 4 * qh + 4  # live k-tiles for this q-half
                    sps_at = {}

                    def emit_scores(kt):
                        sps = [
                            sc_ps.tile([128, 512], F32, tag="sc", name=f"sps{i}")
                            for i in range(2)
                        ]
                        # PE issue is ~216ns/instr regardless of width, so
                        # use one full 512-col matmul pair when both q
                        # halves are live; 256-col only on the causal edge.
                        full = kt <= 4 * qh + 1
                        qsl = (
                            slice(qh * 512, qh * 512 + 512)
                            if full
                            else slice(qh * 512 + 256, qh * 512 + 512)
                        )
                        osl = slice(0, 512) if full else slice(256, 512)
                        ksl = slice(kt * 128, (kt + 1) * 128)
                        for hh in range(2):
                            nc.tensor.matmul(
                                sps[hh][:, osl],
                                kpk[hh][:, ksl],
                                qpk[hh][:, qsl],
                                start=True,
                                stop=False,
                            )
                        for hh in range(2):
                            p0 = hh * 64
                            nc.tensor.matmul(
                                sps[hh][:, osl],
                                ka1[p0 : p0 + 64, ksl],
                                qa1[p0 : p0 + 64, qsl],
                                start=False,
                                stop=True,
                            )
                        # mask only on diagonal-band halves
                        for qbh in range(2):
                            qb = 2 * qh + qbh
                            if kt in (2 * qb, 2 * qb + 1):
                                msl = slice(qbh * 256, qbh * 256 + 256)
                                for hh in range(2):
                                    nc.vector.tensor_add(
                                        sps[hh][:, msl],
                                        sps[hh][:, msl],
                                        mask_sb[:, kt % 2, :],
                                    )
                        # exp -> bf16 attn tiles (dead qb0 half zeroed so
                        # the full-width ctx matmul accumulates one group
                        # per PSUM bank)
                        at = [
                            attn_pool.tile([128, 512], BF, tag="attn", name=f"at{i}")
                            for i in range(2)
                        ]
                        for hh in range(2):
                            if not full:
                                nc.vector.memset(at[hh][:, 0:256], 0.0)
                            nc.scalar.activation(
                                at[hh][:, osl], sps[hh][:, osl], AF.Exp, scale=SCALE
                            )
                        sps_at[kt] = at

                    def emit_ctx(kt):
                        at = sps_at.pop(kt)
                        for hh in range(2):
                            h = 2 * f + hh
                            nc.tensor.matmul(
                                cps[hh][:, :],
                                vnat[:, kt, h, :],
                                at[hh][:, :],
                                start=(kt == 0),
                                stop=(kt == n_kt - 1),
                            )

                    prev = None
                    for kt in range(n_kt):
                        emit_scores(kt)
                        if prev is not None:
                            emit_ctx(prev)
                        prev = kt
                    emit_ctx(prev)

                    # normalize: divide ctx rows by the attn row-sums that
                    # accumulated in psum row DH (sum staged to SBUF for the
                    # fast custom-DVE reciprocal, which is SBUF-only)
                    for hh in range(2):
                        p0 = hh * 64
                        sr = small.tile([1, 512], F32, tag="sr", bufs=2)
                        nc.scalar.copy(sr[:, :], cps[hh][DH : DH + 1, :])
                        inv = small.tile([1, 512], F32, tag="inv", bufs=2)
                        nc.vector.reciprocal_approx_fast(inv[:, :], sr[:, :])
                        inv_bc = small.tile([64, 512], F32, tag="invbc", bufs=2)
                        nc.gpsimd.partition_broadcast(inv_bc[:, :], inv[:, :])
                        nc.vector.tensor_mul(
                            ctxf[p0 : p0 + 64, qh * 512 : (qh + 1) * 512],
                            cps[hh][0:DH, :],
                            inv_bc[:, :],
                        )

                    # stage this q-half (= dest-rank chunk) for the
                    # exchange as soon as it is normalized
                    for s2 in range(2):
                        nc.vector.tensor_mul(
                            cxs[:, qh, s2, :],
                            ctxf[:, qh * 512 : (qh + 1) * 512],
                            sel_sb[:, s2, :],
                        )
                        nc.sync.dma_start(
                            out=cx_in[qh, s2, :, :], in_=cxs[:, qh, s2, :]
                        )

                # -- exchange fold ctx halves with the pair core.
                # AllToAll isn't available for 2-core groups, so emulate it
                # with a ReduceScatter over [dest d][chan-slot s] staging
                # where slot s != own-half is zeroed via the host-provided
                # 0/1 `sel` tensor (x + 0 is exact in bf16). Rank d then
                # receives [ctx_half0, ctx_half1] for its own rows. --
                cx_out = dram.tile(
                    [2, 128, 512], BF, tag=f"cxout{f}", name=f"cxout{f}"
                )
                nc.gpsimd.collective_compute(
                    "ReduceScatter",
                    ALU.add,
                    replica_groups=GROUPS,
                    ins=[cx_in.opt()],
                    outs=[cx_out.opt()],
                )
                for s2 in range(2):
                    nc.sync.dma_start(
                        out=ctx_all[:, s2 * NQF + f, :], in_=cx_out[s2, :, :]
                    )

            emit_vproj()
            emit_proj(0)
            for f in range(NQF):
                if f + 1 < NQF:
                    emit_proj(f + 1)
                emit_attention(f)

            # ---- out-projection over full 1024 channels for own rows ----
            # 8 chains (lt, dh). Chains for lt 0,1,3 are partially
            # accumulated (folds 0-2 contributions + residual) right after
            # fold-3 attention, filling the PE idle window while fold 3's
            # normalize/exchange runs; the fold-3 contributions and the lt2
            # chains run after the last readback.
            early_chains = [(lt, dh) for lt in (0, 1, 3) for dh in range(2)]
            late_chains = [(2, 0), (2, 1)]
            pools = [sc_ps, proj_ps]
            ptags = ["sc", "proj"]
            ops = {}
            for i, ch in enumerate(early_chains):
                ops[ch] = pools[i % 2].tile(
                    [128, 512], F32, tag=ptags[i % 2], name=f"opse{i}"
                )

            def op_mm(ch, cf, start, stop):
                lt, dh = ch
                nc.tensor.matmul(
                    ops[ch][:, :],
                    ctx_all[:, cf, lt * 128 : (lt + 1) * 128],
                    wout_sb[:, cf, dh * 512 : (dh + 1) * 512],
                    start=start,
                    stop=stop,
                )

            def op_eye(ch, stop):
                lt, dh = ch
                nc.tensor.matmul(
                    ops[ch][:, :],
                    eye_sb[:, :],
                    ev_sb[lt][:, dh * 512 : (dh + 1) * 512],
                    start=False,
                    stop=stop,
                )

            for cf in (0, 1, 2, 4, 5, 6):
                for ch in early_chains:
                    op_mm(ch, cf, start=(cf == 0), stop=False)
            for ch in early_chains:
                op_eye(ch, stop=False)
            # ---- late part: fold-3 contributions ----
            for cf in (3, 7):
                for ch in early_chains:
                    op_mm(ch, cf, start=False, stop=(cf == 7))
            for i, ch in enumerate(late_chains):
                ops[ch] = pools[i % 2].tile(
                    [128, 512], F32, tag=ptags[i % 2], name=f"opsl{i}"
                )
            for cf in range(8):
                for ch in late_chains:
                    op_mm(ch, cf, start=(cf == 0), stop=False)
            for ch in late_chains:
                op_eye(ch, stop=True)

            # ---- evict + LayerNorm per row tile ----
            for lt in (0, 1, 3, 2):
                lsl = slice(lt * 128, (lt + 1) * 128)
                xt_ = ln_pool.tile([128, D], F32, tag="x", name=f"x{lt}")
                nc.scalar.copy(xt_[:, 0:512], ops[(lt, 0)][:, :])
                nc.scalar.copy(xt_[:, 512:1024], ops[(lt, 1)][:, :])
                stats = small.tile([128, 2, 6], F32, tag="stats")
                nc.vector.bn_stats(out=stats[:, 0, :], in_=xt_[:, 0:512])
                nc.vector.bn_stats(out=stats[:, 1, :], in_=xt_[:, 512:1024])
                mv = small.tile([128, 2], F32, tag="mv")
                nc.vector.bn_aggr(out=mv[:, :], in_=stats[:, :, :])
                std = small.tile([128, 1], F32, tag="std")
                nc.scalar.activation(std[:, :], mv[:, 1:2], AF.Sqrt, bias=eps_sb[:, :])
                rstd = small.tile([128, 1], F32, tag="rstd")
                nc.vector.reciprocal(rstd[:, :], std[:, :])
                negmb = small.tile([128, 1], F32, tag="negmb")
                nc.vector.scalar_tensor_tensor(
                    negmb[:, :],
                    mv[:, 0:1],
                    -1.0,
                    rstd[:, :],
                    op0=ALU.mult,
                    op1=ALU.mult,
                )
                dacc = small.tile([128, 1], F32, tag="dacc")
                nc.vector.affine_mul_reduce(
                    xt_[:, :],
                    dacc[:, :],
                    xt_[:, :],
                    gb_bc[:, 0, :],
                    scale=rstd[:, :],
                    bias=negmb[:, :],
                )
                nc.vector.tensor_add(xt_[:, :], xt_[:, :], gb_bc[:, 1, :])
                nc.sync.dma_start(out=out[lsl, :], in_=xt_[:, :])

    nc.compile()
    return nc


def _get_nc():
    if "nc" not in _NC_CACHE:
        _NC_CACHE["nc"] = build_nc()
    return _NC_CACHE["nc"]


def kernel(
    e_v, e_a0, e_a1, Wqv, bqv, Wkv, bkv, Wvv, bvv,
    Wqa0, bqa0, Wka0, bka0, Wqa1, bqa1, Wka1, bka1,
    Wout, bout, ln_gamma, ln_beta, attn_mask,
):
    global LAST_RESULT
    f = np.asarray
    e_v, e_a0, e_a1 = f(e_v), f(e_a0), f(e_a1)
    attn_mask = f(attn_mask)
    c32 = lambda a: np.ascontiguousarray(a, dtype=np.float32)
    cbf = lambda a: np.ascontiguousarray(np.asarray(a, dtype=np.float32).astype(BF16))

    wq_full = {"v": f(Wqv), "a0": f(Wqa0), "a1": f(Wqa1)}
    wk_full = {"v": f(Wkv), "a0": f(Wka0), "a1": f(Wka1)}
    bq_full = {"v": f(bqv), "a0": f(bqa0), "a1": f(bqa1)}
    bk_full = {"v": f(bkv), "a0": f(bka0), "a1": f(bka1)}

    xts = {}
    maskABs = {}
    for b in range(B):
        xts[b] = {
            "v": cbf(e_v[b].T),
            "a0": cbf(e_a0[b].T),
            "a1": cbf(e_a1[b].T),
        }
        mT = f(attn_mask[b, 0]).T * (1.0 / SCALE)
        # diagonal-band mask patterns: A = (kt == 2*qb), B = (kt == 2*qb+1)
        maskABs[b] = cbf(np.stack([mT[0:128, 0:256], mT[128:256, 0:256]], axis=1))

    def fold_slice(w, S):
        # [D, C] slice -> [NQF, D, 128] fold-major
        ws = np.asarray(w[:, S], dtype=np.float32)  # [D, C]
        return np.ascontiguousarray(
            ws.reshape(D, NQF, 128).transpose(1, 0, 2).astype(BF16)
        )

    Wout_f = f(Wout).astype(np.float32)
    bout_f = f(bout).astype(np.float32)
    # [128 p, 8 cf, D] with cf = s*4+f mapping Wout rows s*512+f*128+p
    wout8 = np.ascontiguousarray(
        Wout_f.reshape(2, NQF, 128, D).transpose(2, 0, 1, 3).astype(BF16)
    ).reshape(128, 8, D)

    in_maps = []
    for c in range(NCORES):
        b, hh = c // 2, c % 2
        S = slice(hh * C, (hh + 1) * C)
        m = {}
        for s in STREAMS:
            m[f"xt_{s}"] = xts[b][s]
            m[f"wq_{s}"] = fold_slice(wq_full[s], S)
            m[f"wk_{s}"] = fold_slice(wk_full[s], S)
            m[f"bq_{s}"] = c32(bq_full[s][S])
            m[f"bk_{s}"] = c32(bk_full[s][S])
        m["wv"] = cbf(f(Wvv)[:, S])
        m["bv"] = cbf(f(bvv)[S]).reshape(1, C)
        m["wout8"] = wout8
        m["maskAB"] = maskABs[b]
        selv = np.zeros((128, 2, 512), dtype=np.float32)
        selv[:, hh, :] = 1.0
        m["sel"] = cbf(selv)
        m["ev_res"] = cbf(e_v[b, hh * 512 : (hh + 1) * 512, :] + bout_f[None, :])
        m["eye"] = cbf(np.eye(128, dtype=np.float32))
        m["gamma"] = c32(f(ln_gamma)).reshape(1, D)
        m["beta"] = c32(f(ln_beta)).reshape(1, D)
        in_maps.append(m)

    nc = _get_nc()
    trace = bool(os.environ.get("KERNEL_TRACE"))
    res = run_bass_kernel_spmd(
        nc, in_maps, core_ids=list(range(NCORES)), trace=trace
    )
    LAST_RESULT = res

    out_full = np.empty((B, L, D), dtype=np.float32)
    for c in range(NCORES):
        b, hh = c // 2, c % 2
        out_full[b, hh * 512 : (hh + 1) * 512, :] = res.results[c]["out"]
    return out_full


# revision 35
# speedup vs baseline: 1.1644x; 1.1016x over previous
"""DiffMHA (differential multi-head attention) block on 8 TRN2 NeuronCores.

Problem: B=4, L=1024, D=1024, H=16 heads (DH=64). Three input streams
(e_v, e_a0, e_a1); Q/K projections per stream, scores summed across
streams, causal-masked softmax, context from the v-stream values,
out-projection + residual + LayerNorm.

Sharding: (batch, head-half) -> 8 cores. Core c handles batch c//2 and
heads (c%2)*8 .. (c%2)*8+8. Each core computes its 8 heads' Q/K/V
projections (512 of 1024 channels), scores + softmax + context. Fold
context halves (128 channels x 512 rows) are exchanged between the two
cores of a batch via per-fold pairwise AllToAll DURING attention; each
core then runs the out-projection for its own 512 sequence rows with
the full 1024-channel contraction, then residual + LayerNorm locally.
No end-of-kernel collective.

Key optimizations over the v1 kernel:
- Causal skipping: score tiles with kt*128 > qb*256+255 are fully
  masked and skipped entirely (exp == 0 exactly); only diagonal-band
  tiles get the mask add. Attention matmul work drops ~40%.
- Stream packing: q/k of streams v and a0 are packed onto 128
  partitions (64 chans each) at projection-eviction time (partition-
  shifted PSUM->SBUF copies), so their two 64-deep score matmuls fuse
  into one 128-deep matmul; stream a1 stays a 64-deep accumulate.
- PSUM bank alternation: consecutive PE matmuls never accumulate into
  the same PSUM bank back-to-back (V-proj lf pairs, per-head score
  tiles, ctx of 2 heads, out-proj chains are interleaved), which keeps
  the PE at its ~216ns/512-col pipelined rate instead of ~430ns.
- Softmax normalization deferred past ctx accumulation via the extra
  ones-row of V (unchanged), reciprocal+broadcast per (head, q-half).
- DMA order: xt_v + wv first so the PE starts ~10us in, not ~46us.
"""

import os
import sys
import types

import ml_dtypes
import numpy as np

B, L, D, H = 4, 1024, 1024, 16
DH = D // H
HPC = H // 2  # heads per core
C = HPC * DH  # channels per core (512)
SCALE = float(1.0 / np.sqrt(DH))
EPS = 1e-12
NCORES = 8
BF16 = ml_dtypes.bfloat16


def _install_ntff_hook():
    """Recreate antenv.axon_hooks (absent in this image) so
    run_bass_kernel_spmd(trace=True) can capture NTFF profiles."""
    if "antenv.axon_hooks" in sys.modules:
        return
    try:
        from trn_agent_boot.trn_boot import _ntff_profile_via_ctypes

        hook = _ntff_profile_via_ctypes("/opt/axon/libaxon_pjrt.so")
    except Exception:
        hook = None
    mod = types.ModuleType("antenv.axon_hooks")
    mod.get_axon_ntff_profile_hook = lambda: hook
    mod.set_axon_ntff_profile_hook = lambda h: None
    sys.modules["antenv.axon_hooks"] = mod


_install_ntff_hook()

import concourse.bass as bass  # noqa: E402
import concourse.mybir as mybir  # noqa: E402
import concourse.tile as tile  # noqa: E402
from concourse import bacc  # noqa: E402
from concourse.bass_utils import run_bass_kernel_spmd  # noqa: E402

F32 = mybir.dt.float32
BF = mybir.dt.bfloat16
F8 = mybir.dt.float8e4
W8_SCALE = 64.0
AF = mybir.ActivationFunctionType
ALU = mybir.AluOpType

_NC_CACHE = {}
LAST_RESULT = None

NQF = C // 128  # 4 channel folds per stream (2 heads each)
NLT = L // 128  # 8 l-tiles
NDT = D // 128  # 8 d-tiles (contraction)
NKT = L // 128  # 8 k-tiles
NRF = (L // 2) // 128  # 4 row tiles for out-proj/LN
STREAMS = ("v", "a0", "a1")
GROUPS = [[0, 1], [2, 3], [4, 5], [6, 7]]


def build_nc():
    nc = bacc.Bacc("TRN2", target_bir_lowering=False, debug=False, num_devices=NCORES)

    # ---- DRAM parameters (per-core shards, host-prepped) ----
    xtv = nc.declare_dram_parameter("xt_v", [D, L], BF, isOutput=False)
    xt8 = {s: nc.declare_dram_parameter(f"xt8_{s}", [D, L], F8, isOutput=False) for s in STREAMS}
    wq = {s: nc.declare_dram_parameter(f"wq_{s}", [NQF, D, 128], F8, isOutput=False) for s in STREAMS}
    wk = {s: nc.declare_dram_parameter(f"wk_{s}", [NQF, D, 128], F8, isOutput=False) for s in STREAMS}
    wv = nc.declare_dram_parameter("wv", [D, C], BF, isOutput=False)
    wout8 = nc.declare_dram_parameter("wout8", [128, 8, D], BF, isOutput=False)
    bq = {s: nc.declare_dram_parameter(f"bq_{s}", [C], F32, isOutput=False) for s in STREAMS}
    bk = {s: nc.declare_dram_parameter(f"bk_{s}", [C], F32, isOutput=False) for s in STREAMS}
    bv = nc.declare_dram_parameter("bv", [1, C], BF, isOutput=False)
    maskAB = nc.declare_dram_parameter("maskAB", [128, 2, 256], BF, isOutput=False)
    sel = nc.declare_dram_parameter("sel", [128, 2, 512], BF, isOutput=False)
    ev_res = nc.declare_dram_parameter("ev_res", [L // 2, D], BF, isOutput=False)
    eye = nc.declare_dram_parameter("eye", [128, 128], BF, isOutput=False)
    gamma = nc.declare_dram_parameter("gamma", [1, D], F32, isOutput=False)
    beta = nc.declare_dram_parameter("beta", [1, D], F32, isOutput=False)
    out = nc.declare_dram_parameter("out", [L // 2, D], F32, isOutput=True)


    with tile.TileContext(nc) as tc:
        with (
            tc.tile_pool(name="persist", bufs=1) as persist,
            tc.tile_pool(name="xtp", bufs=1) as xtp,
            tc.tile_pool(name="wf", bufs=10) as wf,
            tc.tile_pool(name="qkf", bufs=2) as qkf,
            tc.tile_pool(name="small", bufs=4) as small,
            tc.tile_pool(name="attn", bufs=4) as attn_pool,
            tc.tile_pool(name="ln", bufs=4) as ln_pool,
            tc.tile_pool(name="evp", bufs=4) as evp,
            tc.tile_pool(name="ctxf", bufs=2) as ctxf_pool,
            tc.tile_pool(name="proj_ps", bufs=3, space="PSUM") as proj_ps,
            tc.tile_pool(name="sc_ps", bufs=3, space="PSUM") as sc_ps,
            tc.tile_pool(name="ctx_ps", bufs=2, space="PSUM") as ctx_ps,
            tc.tile_pool(name="dram", bufs=1, space="DRAM") as dram,
        ):
            # ---- persistent SBUF tensors ----
            vnat = persist.tile([128, NLT, HPC, DH + 1], BF, tag="vnat")
            ctx_all = persist.tile([128, 8, L // 2], BF, tag="ctxall")
            mask_sb = persist.tile([128, 2, 256], BF, tag="maskAB")
            ones_b = persist.tile([1, L], BF, tag="ones")
            gb_bc = persist.tile([128, 2, D], F32, tag="gbbc")
            bv_sb = persist.tile([1, C], BF, tag="bvsb")
            wv_sb = persist.tile([128, NDT, C], BF, tag="wvsb")
            wout_sb = persist.tile([128, 8, D], BF, tag="woutsb")
            eps_sb = persist.tile([128, 1], F32, tag="eps")
            eye_sb = persist.tile([128, 128], BF, tag="eye")
            bq_sb = {
                s: persist.tile([128, NQF], F32, tag=f"bq{s}", name=f"bq_sb_{s}")
                for s in STREAMS
            }
            bk_sb = {
                s: persist.tile([128, NQF], F32, tag=f"bk{s}", name=f"bk_sb_{s}")
                for s in STREAMS
            }

            # ---- fold weight loader (lazy, cached) ----
            wf_cache = {}

            def load_wf(s, ff):
                if (s, ff) in wf_cache:
                    return wf_cache[(s, ff)]
                wq_t = wf.tile([128, NDT, 128], F8, tag="w", name=f"wq_{s}{ff}")
                wk_t = wf.tile([128, NDT, 128], F8, tag="w", name=f"wk_{s}{ff}")
                nc.sync.dma_start(
                    out=wq_t[:, :, :],
                    in_=wq[s][ff, :, :].rearrange("(dt p) c -> p dt c", p=128),
                )
                nc.sync.dma_start(
                    out=wk_t[:, :, :],
                    in_=wk[s][ff, :, :].rearrange("(dt p) c -> p dt c", p=128),
                )
                wf_cache[(s, ff)] = (wq_t, wk_t)
                return wq_t, wk_t

            # ---- preamble DMAs, in critical-path order: V-proj deps
            #      first, then fold-0 Q/K weights interleaved with the
            #      remaining embeddings; big late-use tensors (wout, ev,
            #      gamma/beta) are deferred into the fold loop. ----
            xtv_sb = xtp.tile([128, NDT, L], BF, tag="xtv", name="xtv_sb")
            nc.sync.dma_start(
                out=xtv_sb[:, :, :],
                in_=xtv[:, :].rearrange("(dt p) l -> p dt l", p=128),
            )
            nc.sync.dma_start(
                out=wv_sb[:, :, :], in_=wv[:, :].rearrange("(dt p) c -> p dt c", p=128)
            )
            nc.sync.dma_start(out=bv_sb[:, :], in_=bv[:, :])
            xt_sb = {}
            for s in STREAMS:
                t = xtp.tile([128, NDT, L], F8, tag=f"xt8{s}", name=f"xt8_sb_{s}")
                nc.sync.dma_start(
                    out=t[:, :, :],
                    in_=xt8[s][:, :].rearrange("(dt p) l -> p dt l", p=128),
                )
                xt_sb[s] = t
                load_wf(s, 0)

            nc.vector.memset(ones_b[:, :], 1.0)
            nc.vector.memset(eps_sb[:, :], EPS)
            nc.vector.memset(vnat[:, :, :, DH : DH + 1], 1.0)

            nc.sync.dma_start(out=mask_sb[:, :, :], in_=maskAB[:, :, :])
            sel_sb = persist.tile([128, 2, 512], BF, tag="sel")
            nc.sync.dma_start(out=sel_sb[:, :, :], in_=sel[:, :, :])
            nc.sync.dma_start(out=eye_sb[:, :], in_=eye[:, :])
            for s in STREAMS:
                nc.sync.dma_start(
                    out=bq_sb[s][:, :], in_=bq[s][:].rearrange("(f p) -> p f", p=128)
                )
                nc.sync.dma_start(
                    out=bk_sb[s][:, :], in_=bk[s][:].rearrange("(f p) -> p f", p=128)
                )
            ev_sb = []

            def emit_vproj():
                # ---- V projection: natural [l, c] layout + ones column.
                #      lf pairs interleaved so consecutive matmuls alternate
                #      PSUM banks. ----
                for pair in range(NLT // 2):
                    lf0, lf1 = 2 * pair, 2 * pair + 1
                    psA = sc_ps.tile([128, C], F32, tag="sc")
                    psB = sc_ps.tile([128, C], F32, tag="sc")
                    for dt in range(NDT):
                        for lf, ps in ((lf0, psA), (lf1, psB)):
                            nc.tensor.matmul(
                                ps[:, :],
                                xtv_sb[:, dt, lf * 128 : (lf + 1) * 128],
                                wv_sb[:, dt, :],
                                start=(dt == 0),
                                stop=False,
                            )
                    for lf, ps in ((lf0, psA), (lf1, psB)):
                        nc.tensor.matmul(
                            ps[:, :],
                            ones_b[:, lf * 128 : (lf + 1) * 128],
                            bv_sb[:, :],
                            start=False,
                            stop=True,
                        )
                    nc.scalar.copy(vnat[:, lf0, :, 0:DH], psA[:, :])
                    nc.scalar.copy(vnat[:, lf1, :, 0:DH], psB[:, :])

            # ---- fold-major main loop. Fold f+1's projections are
            #      emitted BEFORE fold f's attention so the packed-Q/K
            #      eviction latency hides under attention compute. ----
            fold_tiles = {}

            def emit_proj(f):
                # packed tiles: partitions [0:64] = stream v chans of the
                # head, [64:128] = stream a0 chans; a1 keeps fold layout.
                qpk = [
                    qkf.tile([128, L], BF, tag=f"qpk{hh}", name=f"qpk{hh}_{f}")
                    for hh in range(2)
                ]
                kpk = [
                    qkf.tile([128, L], BF, tag=f"kpk{hh}", name=f"kpk{hh}_{f}")
                    for hh in range(2)
                ]
                qa1 = qkf.tile([128, L], BF, tag="qa1", name=f"qa1_{f}")
                ka1 = qkf.tile([128, L], BF, tag="ka1", name=f"ka1_{f}")
                for s in STREAMS:
                    wq_t, wk_t = load_wf(s, f)
                    for w_t, b_t, pk, a1t in (
                        (wq_t, bq_sb[s], qpk, qa1),
                        (wk_t, bk_sb[s], kpk, ka1),
                    ):
                        ps = [
                            proj_ps.tile([128, 512], F32, tag="proj", name=f"pp{lh}")
                            for lh in range(2)
                        ]
                        for dt2 in range(NDT // 2):
                            for lh in range(2):
                                nc.tensor.matmul(
                                    ps[lh][:, :],
                                    w_t[:, 2 * dt2 : 2 * dt2 + 2, :],
                                    xt_sb[s][
                                        :,
                                        2 * dt2 : 2 * dt2 + 2,
                                        lh * 512 : (lh + 1) * 512,
                                    ],
                                    start=(dt2 == 0),
                                    stop=(dt2 == NDT // 2 - 1),
                                    perf_mode=mybir.MatmulPerfMode.DoubleRow,
                                )
                        for lh in range(2):
                            lsl = slice(lh * 512, (lh + 1) * 512)
                            if s == "a1":
                                nc.scalar.activation(
                                    a1t[:, lsl], ps[lh][:, :], AF.Identity,
                                    bias=b_t[:, f : f + 1],
                                    scale=1.0 / W8_SCALE,
                                )
                            else:
                                off = 0 if s == "v" else 64
                                for hh in range(2):
                                    nc.scalar.activation(
                                        pk[hh][off : off + 64, lsl],
                                        ps[lh][hh * 64 : hh * 64 + 64, :],
                                        AF.Identity,
                                        bias=b_t[hh * 64 : hh * 64 + 64, f : f + 1],
                                        scale=1.0 / W8_SCALE,
                                    )
                fold_tiles[f] = (qpk, kpk, qa1, ka1)

            def emit_attention(f):
                qpk, kpk, qa1, ka1 = fold_tiles.pop(f)

                # stage late-use loads here so they don't compete with the
                # critical-path preamble/projection DMAs
                if f == 0:
                    nc.sync.dma_start(out=wout_sb[:, :, :], in_=wout8[:, :, :])
                if f == 1:
                    for rf in range(NRF):
                        t = evp.tile([128, D], BF, tag="ev", name=f"ev{rf}")
                        nc.sync.dma_start(
                            out=t[:, :], in_=ev_res[rf * 128 : (rf + 1) * 128, :]
                        )
                        ev_sb.append(t)
                if f == 2:
                    gsb = small.tile([1, D], F32, tag="gsb", bufs=1)
                    bsb = small.tile([1, D], F32, tag="bsb", bufs=1)
                    nc.sync.dma_start(out=gsb[:, :], in_=gamma[:, :])
                    nc.sync.dma_start(out=bsb[:, :], in_=beta[:, :])
                    nc.gpsimd.partition_broadcast(gb_bc[:, 0, :], gsb[:, :])
                    nc.gpsimd.partition_broadcast(gb_bc[:, 1, :], bsb[:, :])

                ctxf = ctxf_pool.tile([128, L], BF, tag="ctxf", name=f"ctxf{f}")
                cxs = ctxf_pool.tile(
                    [128, 2, 2, 512], BF, tag="cxs", name=f"cxs{f}", bufs=1
                )
                cx_in = dram.tile(
                    [2, 2, 128, 512], BF, tag=f"cxin{f}", name=f"cxin{f}"
                )
                for qh in range(2):
                    cps = [
                        ctx_ps.tile([DH + 1, 512], F32, tag="ctx", name=f"cps{i}")
                        for i in range(2)
                    ]
                    n_kt = 4 * qh + 4  # live k-tiles for this q-half
                    sps_at = {}

                    def emit_scores(kt):
                        sps = [
                            sc_ps.tile([128, 512], F32, tag="sc", name=f"sps{i}")
                            for i in range(2)
                        ]
                        # PE issue is ~216ns/instr regardless of width, so
                        # use one full 512-col matmul pair when both q
                        # halves are live; 256-col only on the causal edge.
                        full = kt <= 4 * qh + 1
                        qsl = (
                            slice(qh * 512, qh * 512 + 512)
                            if full
                            else slice(qh * 512 + 256, qh * 512 + 512)
                        )
                        osl = slice(0, 512) if full else slice(256, 512)
                        ksl = slice(kt * 128, (kt + 1) * 128)
                        for hh in range(2):
                            nc.tensor.matmul(
                                sps[hh][:, osl],
                                kpk[hh][:, ksl],
                                qpk[hh][:, qsl],
                                start=True,
                                stop=False,
                            )
                        for hh in range(2):
                            p0 = hh * 64
                            nc.tensor.matmul(
                                sps[hh][:, osl],
                                ka1[p0 : p0 + 64, ksl],
                                qa1[p0 : p0 + 64, qsl],
                                start=False,
                                stop=True,
                            )
                        # mask only on diagonal-band halves
                        for qbh in range(2):
                            qb = 2 * qh + qbh
                            if kt in (2 * qb, 2 * qb + 1):
                                msl = slice(qbh * 256, qbh * 256 + 256)
                                for hh in range(2):
                                    nc.vector.tensor_add(
                                        sps[hh][:, msl],
                                        sps[hh][:, msl],
                                        mask_sb[:, kt % 2, :],
                                    )
                        # exp -> bf16 attn tiles (dead qb0 half zeroed so
                        # the full-width ctx matmul accumulates one group
                        # per PSUM bank)
                        at = [
                            attn_pool.tile([128, 512], BF, tag="attn", name=f"at{i}")
                            for i in range(2)
                        ]
                        for hh in range(2):
                            if not full:
                                nc.vector.memset(at[hh][:, 0:256], 0.0)
                            nc.scalar.activation(
                                at[hh][:, osl], sps[hh][:, osl], AF.Exp, scale=SCALE
                            )
                        sps_at[kt] = at

                    def emit_ctx(kt):
                        at = sps_at.pop(kt)
                        for hh in range(2):
                            h = 2 * f + hh
                            nc.tensor.matmul(
                                cps[hh][:, :],
                                vnat[:, kt, h, :],
                                at[hh][:, :],
                                start=(kt == 0),
                                stop=(kt == n_kt - 1),
                            )

                    prev = None
                    for kt in range(n_kt):
                        emit_scores(kt)
                        if prev is not None:
                            emit_ctx(prev)
                        prev = kt
                    emit_ctx(prev)

                    # normalize: divide ctx rows by the attn row-sums that
                    # accumulated in psum row DH (sum staged to SBUF for the
                    # fast custom-DVE reciprocal, which is SBUF-only)
                    for hh in range(2):
                        p0 = hh * 64
                        sr = small.tile([1, 512], F32, tag="sr", bufs=2)
                        nc.scalar.copy(sr[:, :], cps[hh][DH : DH + 1, :])
                        inv = small.tile([1, 512], F32, tag="inv", bufs=2)
                        nc.vector.reciprocal_approx_fast(inv[:, :], sr[:, :])
                        inv_bc = small.tile([64, 512], F32, tag="invbc", bufs=2)
                        nc.gpsimd.partition_broadcast(inv_bc[:, :], inv[:, :])
                        nc.vector.tensor_mul(
                            ctxf[p0 : p0 + 64, qh * 512 : (qh + 1) * 512],
                            cps[hh][0:DH, :],
                            inv_bc[:, :],
                        )

                    # stage this q-half (= dest-rank chunk) for the
                    # exchange as soon as it is normalized
                    for s2 in range(2):
                        nc.vector.tensor_mul(
                            cxs[:, qh, s2, :],
                            ctxf[:, qh * 512 : (qh + 1) * 512],
                            sel_sb[:, s2, :],
                        )
                        nc.sync.dma_start(
                            out=cx_in[qh, s2, :, :], in_=cxs[:, qh, s2, :]
                        )

                # -- exchange fold ctx halves with the pair core.
                # AllToAll isn't available for 2-core groups, so emulate it
                # with a ReduceScatter over [dest d][chan-slot s] staging
                # where slot s != own-half is zeroed via the host-provided
                # 0/1 `sel` tensor (x + 0 is exact in bf16). Rank d then
                # receives [ctx_half0, ctx_half1] for its own rows. --
                cx_out = dram.tile(
                    [2, 128, 512], BF, tag=f"cxout{f}", name=f"cxout{f}"
                )
                nc.gpsimd.collective_compute(
                    "ReduceScatter",
                    ALU.add,
                    replica_groups=GROUPS,
                    ins=[cx_in.opt()],
                    outs=[cx_out.opt()],
                )
                for s2 in range(2):
                    nc.sync.dma_start(
                        out=ctx_all[:, s2 * NQF + f, :], in_=cx_out[s2, :, :]
                    )

            emit_vproj()
            emit_proj(0)
            for f in range(NQF):
                if f + 1 < NQF:
                    emit_proj(f + 1)
                emit_attention(f)

            # ---- out-projection over full 1024 channels for own rows ----
            # 8 chains (lt, dh). Chains for lt 0,1,3 are partially
            # accumulated (folds 0-2 contributions + residual) right after
            # fold-3 attention, filling the PE idle window while fold 3's
            # normalize/exchange runs; the fold-3 contributions and the lt2
            # chains run after the last readback.
            early_chains = [(lt, dh) for lt in (0, 1, 3) for dh in range(2)]
            late_chains = [(2, 0), (2, 1)]
            pools = [sc_ps, proj_ps]
            ptags = ["sc", "proj"]
            ops = {}
            for i, ch in enumerate(early_chains):
                ops[ch] = pools[i % 2].tile(
                    [128, 512], F32, tag=ptags[i % 2], name=f"opse{i}"
                )

            def op_mm(ch, cf, start, stop):
                lt, dh = ch
                nc.tensor.matmul(
                    ops[ch][:, :],
                    ctx_all[:, cf, lt * 128 : (lt + 1) * 128],
                    wout_sb[:, cf, dh * 512 : (dh + 1) * 512],
                    start=start,
                    stop=stop,
                )

            def op_eye(ch, stop):
                lt, dh = ch
                nc.tensor.matmul(
                    ops[ch][:, :],
                    eye_sb[:, :],
                    ev_sb[lt][:, dh * 512 : (dh + 1) * 512],
                    start=False,
                    stop=stop,
                )

            for cf in (0, 1, 2, 4, 5, 6):
                for ch in early_chains:
                    op_mm(ch, cf, start=(cf == 0), stop=False)
            for ch in early_chains:
                op_eye(ch, stop=False)
            # ---- late part: fold-3 contributions ----
            for cf in (3, 7):
                for ch in early_chains:
                    op_mm(ch, cf, start=False, stop=(cf == 7))
            for i, ch in enumerate(late_chains):
                ops[ch] = pools[i % 2].tile(
                    [128, 512], F32, tag=ptags[i % 2], name=f"opsl{i}"
                )
            for cf in range(8):
                for ch in late_chains:
                    op_mm(ch, cf, start=(cf == 0), stop=False)
            for ch in late_chains:
                op_eye(ch, stop=True)

            # ---- evict + LayerNorm per row tile ----
            for lt in (0, 1, 3, 2):
                lsl = slice(lt * 128, (lt + 1) * 128)
                xt_ = ln_pool.tile([128, D], F32, tag="x", name=f"x{lt}")
                nc.scalar.copy(xt_[:, 0:512], ops[(lt, 0)][:, :])
                nc.scalar.copy(xt_[:, 512:1024], ops[(lt, 1)][:, :])
                stats = small.tile([128, 2, 6], F32, tag="stats")
                nc.vector.bn_stats(out=stats[:, 0, :], in_=xt_[:, 0:512])
                nc.vector.bn_stats(out=stats[:, 1, :], in_=xt_[:, 512:1024])
                mv = small.tile([128, 2], F32, tag="mv")
                nc.vector.bn_aggr(out=mv[:, :], in_=stats[:, :, :])
                std = small.tile([128, 1], F32, tag="std")
                nc.scalar.activation(std[:, :], mv[:, 1:2], AF.Sqrt, bias=eps_sb[:, :])
                rstd = small.tile([128, 1], F32, tag="rstd")
                nc.vector.reciprocal(rstd[:, :], std[:, :])
                negmb = small.tile([128, 1], F32, tag="negmb")
                nc.vector.scalar_tensor_tensor(
                    negmb[:, :],
                    mv[:, 0:1],
                    -1.0,
                    rstd[:, :],
                    op0=ALU.mult,
                    op1=ALU.mult,
                )
                dacc = small.tile([128, 1], F32, tag="dacc")
                nc.vector.affine_mul_reduce(
                    xt_[:, :],
                    dacc[:, :],
                    xt_[:, :],
                    gb_bc[:, 0, :],
                    scale=rstd[:, :],
                    bias=negmb[:, :],
                )
                nc.vector.tensor_add(xt_[:, :], xt_[:, :], gb_bc[:, 1, :])
                nc.sync.dma_start(out=out[lsl, :], in_=xt_[:, :])

    nc.compile()
    return nc


def _get_nc():
    if "nc" not in _NC_CACHE:
        _NC_CACHE["nc"] = build_nc()
    return _NC_CACHE["nc"]


def kernel(
    e_v, e_a0, e_a1, Wqv, bqv, Wkv, bkv, Wvv, bvv,
    Wqa0, bqa0, Wka0, bka0, Wqa1, bqa1, Wka1, bka1,
    Wout, bout, ln_gamma, ln_beta, attn_mask,
):
    global LAST_RESULT
    f = np.asarray
    e_v, e_a0, e_a1 = f(e_v), f(e_a0), f(e_a1)
    attn_mask = f(attn_mask)
    c32 = lambda a: np.ascontiguousarray(a, dtype=np.float32)
    cbf = lambda a: np.ascontiguousarray(np.asarray(a, dtype=np.float32).astype(BF16))
    F8NP = ml_dtypes.float8_e4m3
    cf8 = lambda a: np.ascontiguousarray(np.asarray(a, dtype=np.float32).astype(F8NP))

    wq_full = {"v": f(Wqv), "a0": f(Wqa0), "a1": f(Wqa1)}
    wk_full = {"v": f(Wkv), "a0": f(Wka0), "a1": f(Wka1)}
    bq_full = {"v": f(bqv), "a0": f(bqa0), "a1": f(bqa1)}
    bk_full = {"v": f(bkv), "a0": f(bka0), "a1": f(bka1)}

    xts = {}
    xt8s = {}
    maskABs = {}
    ev_all = {"v": e_v, "a0": e_a0, "a1": e_a1}
    for b in range(B):
        xts[b] = {"v": cbf(e_v[b].T)}
        xt8s[b] = {s: cf8(ev_all[s][b].T) for s in STREAMS}
        mT = f(attn_mask[b, 0]).T * (1.0 / SCALE)
        # diagonal-band mask patterns: A = (kt == 2*qb), B = (kt == 2*qb+1)
        maskABs[b] = cbf(np.stack([mT[0:128, 0:256], mT[128:256, 0:256]], axis=1))

    def fold_slice(w, S):
        # [D, C] slice -> [NQF, D, 128] fold-major, fp8 with x64 pre-scale
        # (descaled in the projection eviction) to stay clear of e4m3
        # subnormals
        ws = np.asarray(w[:, S], dtype=np.float32) * 64.0  # [D, C]
        return np.ascontiguousarray(
            ws.reshape(D, NQF, 128).transpose(1, 0, 2).astype(F8NP)
        )

    Wout_f = f(Wout).astype(np.float32)
    bout_f = f(bout).astype(np.float32)
    # [128 p, 8 cf, D] with cf = s*4+f mapping Wout rows s*512+f*128+p
    wout8 = np.ascontiguousarray(
        Wout_f.reshape(2, NQF, 128, D).transpose(2, 0, 1, 3).astype(BF16)
    ).reshape(128, 8, D)

    in_maps = []
    for c in range(NCORES):
        b, hh = c // 2, c % 2
        S = slice(hh * C, (hh + 1) * C)
        m = {}
        m["xt_v"] = xts[b]["v"]
        for s in STREAMS:
            m[f"xt8_{s}"] = xt8s[b][s]
            m[f"wq_{s}"] = fold_slice(wq_full[s], S)
            m[f"wk_{s}"] = fold_slice(wk_full[s], S)
            m[f"bq_{s}"] = c32(bq_full[s][S])
            m[f"bk_{s}"] = c32(bk_full[s][S])
        m["wv"] = cbf(f(Wvv)[:, S])
        m["bv"] = cbf(f(bvv)[S]).reshape(1, C)
        m["wout8"] = wout8
        m["maskAB"] = maskABs[b]
        selv = np.zeros((128, 2, 512), dtype=np.float32)
        selv[:, hh, :] = 1.0
        m["sel"] = cbf(selv)
        m["ev_res"] = cbf(e_v[b, hh * 512 : (hh + 1) * 512, :] + bout_f[None, :])
        m["eye"] = cbf(np.eye(128, dtype=np.float32))
        m["gamma"] = c32(f(ln_gamma)).reshape(1, D)
        m["beta"] = c32(f(ln_beta)).reshape(1, D)
        in_maps.append(m)

    nc = _get_nc()
    trace = bool(os.environ.get("KERNEL_TRACE"))
    res = run_bass_kernel_spmd(
        nc, in_maps, core_ids=list(range(NCORES)), trace=trace
    )
    LAST_RESULT = res

    out_full = np.empty((B, L, D), dtype=np.float32)
    for c in range(NCORES):
        b, hh = c // 2, c % 2
        out_full[b, hh * 512 : (hh + 1) * 512, :] = res.results[c]["out"]
    return out_full


# revision 36
# speedup vs baseline: 1.2193x; 1.0471x over previous
"""DiffMHA (differential multi-head attention) block on 8 TRN2 NeuronCores.

Problem: B=4, L=1024, D=1024, H=16 heads (DH=64). Three input streams
(e_v, e_a0, e_a1); Q/K projections per stream, scores summed across
streams, causal-masked softmax, context from the v-stream values,
out-projection + residual + LayerNorm.

Sharding: (batch, head-half) -> 8 cores. Core c handles batch c//2 and
heads (c%2)*8 .. (c%2)*8+8. Each core computes its 8 heads' Q/K/V
projections (512 of 1024 channels), scores + softmax + context. Fold
context halves (128 channels x 512 rows) are exchanged between the two
cores of a batch via per-fold pairwise AllToAll DURING attention; each
core then runs the out-projection for its own 512 sequence rows with
the full 1024-channel contraction, then residual + LayerNorm locally.
No end-of-kernel collective.

Key optimizations over the v1 kernel:
- Causal skipping: score tiles with kt*128 > qb*256+255 are fully
  masked and skipped entirely (exp == 0 exactly); only diagonal-band
  tiles get the mask add. Attention matmul work drops ~40%.
- Stream packing: q/k of streams v and a0 are packed onto 128
  partitions (64 chans each) at projection-eviction time (partition-
  shifted PSUM->SBUF copies), so their two 64-deep score matmuls fuse
  into one 128-deep matmul; stream a1 stays a 64-deep accumulate.
- PSUM bank alternation: consecutive PE matmuls never accumulate into
  the same PSUM bank back-to-back (V-proj lf pairs, per-head score
  tiles, ctx of 2 heads, out-proj chains are interleaved), which keeps
  the PE at its ~216ns/512-col pipelined rate instead of ~430ns.
- Softmax normalization deferred past ctx accumulation via the extra
  ones-row of V (unchanged), reciprocal+broadcast per (head, q-half).
- DMA order: xt_v + wv first so the PE starts ~10us in, not ~46us.
"""

import os
import sys
import types

import ml_dtypes
import numpy as np

B, L, D, H = 4, 1024, 1024, 16
DH = D // H
HPC = H // 2  # heads per core
C = HPC * DH  # channels per core (512)
SCALE = float(1.0 / np.sqrt(DH))
EPS = 1e-12
NCORES = 8
BF16 = ml_dtypes.bfloat16


def _install_ntff_hook():
    """Recreate antenv.axon_hooks (absent in this image) so
    run_bass_kernel_spmd(trace=True) can capture NTFF profiles."""
    if "antenv.axon_hooks" in sys.modules:
        return
    try:
        from trn_agent_boot.trn_boot import _ntff_profile_via_ctypes

        hook = _ntff_profile_via_ctypes("/opt/axon/libaxon_pjrt.so")
    except Exception:
        hook = None
    mod = types.ModuleType("antenv.axon_hooks")
    mod.get_axon_ntff_profile_hook = lambda: hook
    mod.set_axon_ntff_profile_hook = lambda h: None
    sys.modules["antenv.axon_hooks"] = mod


_install_ntff_hook()

import concourse.bass as bass  # noqa: E402
import concourse.mybir as mybir  # noqa: E402
import concourse.tile as tile  # noqa: E402
from concourse import bacc  # noqa: E402
from concourse.bass_utils import run_bass_kernel_spmd  # noqa: E402

F32 = mybir.dt.float32
BF = mybir.dt.bfloat16
F8 = mybir.dt.float8e4
W8_SCALE = 64.0
AF = mybir.ActivationFunctionType
ALU = mybir.AluOpType

_NC_CACHE = {}
LAST_RESULT = None

NQF = C // 128  # 4 channel folds per stream (2 heads each)
NLT = L // 128  # 8 l-tiles
NDT = D // 128  # 8 d-tiles (contraction)
NKT = L // 128  # 8 k-tiles
NRF = (L // 2) // 128  # 4 row tiles for out-proj/LN
STREAMS = ("v", "a0", "a1")
GROUPS = [[0, 1], [2, 3], [4, 5], [6, 7]]


def build_nc():
    nc = bacc.Bacc("TRN2", target_bir_lowering=False, debug=False, num_devices=NCORES)

    # ---- DRAM parameters (per-core shards, host-prepped) ----
    xtv = nc.declare_dram_parameter("xt_v", [D, L], BF, isOutput=False)
    xt8 = {s: nc.declare_dram_parameter(f"xt8_{s}", [D, L], F8, isOutput=False) for s in STREAMS}
    wq = {s: nc.declare_dram_parameter(f"wq_{s}", [NQF, D, 128], F8, isOutput=False) for s in STREAMS}
    wk = {s: nc.declare_dram_parameter(f"wk_{s}", [NQF, D, 128], F8, isOutput=False) for s in STREAMS}
    wv = nc.declare_dram_parameter("wv", [D, C], BF, isOutput=False)
    wout8 = nc.declare_dram_parameter("wout8", [128, 8, D], BF, isOutput=False)
    bq = {s: nc.declare_dram_parameter(f"bq_{s}", [C], F32, isOutput=False) for s in STREAMS}
    bk = {s: nc.declare_dram_parameter(f"bk_{s}", [C], F32, isOutput=False) for s in STREAMS}
    bv = nc.declare_dram_parameter("bv", [1, C], BF, isOutput=False)
    maskAB = nc.declare_dram_parameter("maskAB", [128, 2, 256], BF, isOutput=False)
    sel = nc.declare_dram_parameter("sel", [128, 2, 512], BF, isOutput=False)
    ev_res = nc.declare_dram_parameter("ev_res", [L // 2, D], BF, isOutput=False)
    eye = nc.declare_dram_parameter("eye", [128, 128], BF, isOutput=False)
    gamma = nc.declare_dram_parameter("gamma", [1, D], F32, isOutput=False)
    beta = nc.declare_dram_parameter("beta", [1, D], F32, isOutput=False)
    out = nc.declare_dram_parameter("out", [L // 2, D], F32, isOutput=True)


    with tile.TileContext(nc) as tc:
        with (
            tc.tile_pool(name="persist", bufs=1) as persist,
            tc.tile_pool(name="xtp", bufs=1) as xtp,
            tc.tile_pool(name="wf", bufs=10) as wf,
            tc.tile_pool(name="qkf", bufs=2) as qkf,
            tc.tile_pool(name="small", bufs=4) as small,
            tc.tile_pool(name="attn", bufs=4) as attn_pool,
            tc.tile_pool(name="ln", bufs=4) as ln_pool,
            tc.tile_pool(name="evp", bufs=4) as evp,
            tc.tile_pool(name="ctxf", bufs=2) as ctxf_pool,
            tc.tile_pool(name="proj_ps", bufs=3, space="PSUM") as proj_ps,
            tc.tile_pool(name="sc_ps", bufs=3, space="PSUM") as sc_ps,
            tc.tile_pool(name="ctx_ps", bufs=2, space="PSUM") as ctx_ps,
            tc.tile_pool(name="dram", bufs=1, space="DRAM") as dram,
        ):
            # ---- persistent SBUF tensors ----
            vnat = persist.tile([128, NLT, HPC, DH + 1], BF, tag="vnat")
            ctx_all = persist.tile([128, 8, L // 2], BF, tag="ctxall")
            mask_sb = persist.tile([128, 2, 256], BF, tag="maskAB")
            ones_b = persist.tile([1, L], BF, tag="ones")
            gb_bc = persist.tile([128, 2, D], F32, tag="gbbc")
            bv_sb = persist.tile([1, C], BF, tag="bvsb")
            wv_sb = persist.tile([128, NDT, C], BF, tag="wvsb")
            wout_sb = persist.tile([128, 8, D], BF, tag="woutsb")
            eps_sb = persist.tile([128, 1], F32, tag="eps")
            eye_sb = persist.tile([128, 128], BF, tag="eye")
            bq_sb = {
                s: persist.tile([128, NQF], F32, tag=f"bq{s}", name=f"bq_sb_{s}")
                for s in STREAMS
            }
            bk_sb = {
                s: persist.tile([128, NQF], F32, tag=f"bk{s}", name=f"bk_sb_{s}")
                for s in STREAMS
            }

            # ---- fold weight loader (lazy, cached) ----
            wf_cache = {}

            def load_wf(s, ff):
                if (s, ff) in wf_cache:
                    return wf_cache[(s, ff)]
                wq_t = wf.tile([128, NDT, 128], F8, tag="w", name=f"wq_{s}{ff}")
                wk_t = wf.tile([128, NDT, 128], F8, tag="w", name=f"wk_{s}{ff}")
                nc.sync.dma_start(
                    out=wq_t[:, :, :],
                    in_=wq[s][ff, :, :].rearrange("(dt p) c -> p dt c", p=128),
                )
                nc.sync.dma_start(
                    out=wk_t[:, :, :],
                    in_=wk[s][ff, :, :].rearrange("(dt p) c -> p dt c", p=128),
                )
                wf_cache[(s, ff)] = (wq_t, wk_t)
                return wq_t, wk_t

            # ---- preamble DMAs, in critical-path order: V-proj deps
            #      first, then fold-0 Q/K weights interleaved with the
            #      remaining embeddings; big late-use tensors (wout, ev,
            #      gamma/beta) are deferred into the fold loop. ----
            xtv_sb = xtp.tile([128, NDT, L], BF, tag="xtv", name="xtv_sb")
            nc.sync.dma_start(
                out=xtv_sb[:, :, :],
                in_=xtv[:, :].rearrange("(dt p) l -> p dt l", p=128),
            )
            nc.sync.dma_start(
                out=wv_sb[:, :, :], in_=wv[:, :].rearrange("(dt p) c -> p dt c", p=128)
            )
            nc.sync.dma_start(out=bv_sb[:, :], in_=bv[:, :])
            xt_sb = {}
            for s in STREAMS:
                t = xtp.tile([128, NDT, L], F8, tag=f"xt8{s}", name=f"xt8_sb_{s}")
                nc.sync.dma_start(
                    out=t[:, :, :],
                    in_=xt8[s][:, :].rearrange("(dt p) l -> p dt l", p=128),
                )
                xt_sb[s] = t
                load_wf(s, 0)

            nc.vector.memset(ones_b[:, :], 1.0)
            nc.vector.memset(eps_sb[:, :], EPS)
            nc.vector.memset(vnat[:, :, :, DH : DH + 1], 1.0)

            nc.sync.dma_start(out=mask_sb[:, :, :], in_=maskAB[:, :, :])
            sel_sb = persist.tile([128, 2, 512], BF, tag="sel")
            nc.sync.dma_start(out=sel_sb[:, :, :], in_=sel[:, :, :])
            nc.sync.dma_start(out=eye_sb[:, :], in_=eye[:, :])
            for s in STREAMS:
                nc.sync.dma_start(
                    out=bq_sb[s][:, :], in_=bq[s][:].rearrange("(f p) -> p f", p=128)
                )
                nc.sync.dma_start(
                    out=bk_sb[s][:, :], in_=bk[s][:].rearrange("(f p) -> p f", p=128)
                )
            ev_sb = []

            def emit_vproj():
                # ---- V projection: natural [l, c] layout + ones column.
                #      lf pairs interleaved so consecutive matmuls alternate
                #      PSUM banks. ----
                for pair in range(NLT // 2):
                    lf0, lf1 = 2 * pair, 2 * pair + 1
                    psA = sc_ps.tile([128, C], F32, tag="sc")
                    psB = sc_ps.tile([128, C], F32, tag="sc")
                    for dt in range(NDT):
                        for lf, ps in ((lf0, psA), (lf1, psB)):
                            nc.tensor.matmul(
                                ps[:, :],
                                xtv_sb[:, dt, lf * 128 : (lf + 1) * 128],
                                wv_sb[:, dt, :],
                                start=(dt == 0),
                                stop=False,
                            )
                    for lf, ps in ((lf0, psA), (lf1, psB)):
                        nc.tensor.matmul(
                            ps[:, :],
                            ones_b[:, lf * 128 : (lf + 1) * 128],
                            bv_sb[:, :],
                            start=False,
                            stop=True,
                        )
                    nc.scalar.copy(vnat[:, lf0, :, 0:DH], psA[:, :])
                    nc.scalar.copy(vnat[:, lf1, :, 0:DH], psB[:, :])

            # ---- fold-major main loop. Fold f+1's projections are
            #      emitted BEFORE fold f's attention so the packed-Q/K
            #      eviction latency hides under attention compute. ----
            fold_tiles = {}

            def emit_proj(f):
                # packed tiles: partitions [0:64] = stream v chans of the
                # head, [64:128] = stream a0 chans; a1 keeps fold layout.
                qpk = [
                    qkf.tile([128, 2, L], F8, tag=f"qpk{hh}", name=f"qpk{hh}_{f}")
                    for hh in range(2)
                ]
                kpk = [
                    qkf.tile([128, 2, L], F8, tag=f"kpk{hh}", name=f"kpk{hh}_{f}")
                    for hh in range(2)
                ]
                # zero the unused upper half of the a1 slot so the packed
                # 256-deep DoubleRow contraction adds exact zeros there
                for t8 in (qpk[0], qpk[1], kpk[0], kpk[1]):
                    nc.vector.memset(t8[64:128, 1, :], 0.0)
                for s in STREAMS:
                    wq_t, wk_t = load_wf(s, f)
                    for w_t, b_t, pk in (
                        (wq_t, bq_sb[s], qpk),
                        (wk_t, bk_sb[s], kpk),
                    ):
                        ps = [
                            proj_ps.tile([128, 512], F32, tag="proj", name=f"pp{lh}")
                            for lh in range(2)
                        ]
                        for dt2 in range(NDT // 2):
                            for lh in range(2):
                                nc.tensor.matmul(
                                    ps[lh][:, :],
                                    w_t[:, 2 * dt2 : 2 * dt2 + 2, :],
                                    xt_sb[s][
                                        :,
                                        2 * dt2 : 2 * dt2 + 2,
                                        lh * 512 : (lh + 1) * 512,
                                    ],
                                    start=(dt2 == 0),
                                    stop=(dt2 == NDT // 2 - 1),
                                    perf_mode=mybir.MatmulPerfMode.DoubleRow,
                                )
                        for lh in range(2):
                            lsl = slice(lh * 512, (lh + 1) * 512)
                            # v -> slot0[0:64], a0 -> slot0[64:128],
                            # a1 -> slot1[0:64] (slot1[64:128] is zero)
                            slot, off = {"v": (0, 0), "a0": (0, 64), "a1": (1, 0)}[s]
                            for hh in range(2):
                                nc.scalar.activation(
                                    pk[hh][off : off + 64, slot, lsl],
                                    ps[lh][hh * 64 : hh * 64 + 64, :],
                                    AF.Identity,
                                    bias=b_t[hh * 64 : hh * 64 + 64, f : f + 1],
                                    scale=1.0 / W8_SCALE,
                                )
                fold_tiles[f] = (qpk, kpk)

            def emit_attention(f):
                qpk, kpk = fold_tiles.pop(f)

                # stage late-use loads here so they don't compete with the
                # critical-path preamble/projection DMAs
                if f == 0:
                    nc.sync.dma_start(out=wout_sb[:, :, :], in_=wout8[:, :, :])
                if f == 1:
                    for rf in range(NRF):
                        t = evp.tile([128, D], BF, tag="ev", name=f"ev{rf}")
                        nc.sync.dma_start(
                            out=t[:, :], in_=ev_res[rf * 128 : (rf + 1) * 128, :]
                        )
                        ev_sb.append(t)
                if f == 2:
                    gsb = small.tile([1, D], F32, tag="gsb", bufs=1)
                    bsb = small.tile([1, D], F32, tag="bsb", bufs=1)
                    nc.sync.dma_start(out=gsb[:, :], in_=gamma[:, :])
                    nc.sync.dma_start(out=bsb[:, :], in_=beta[:, :])
                    nc.gpsimd.partition_broadcast(gb_bc[:, 0, :], gsb[:, :])
                    nc.gpsimd.partition_broadcast(gb_bc[:, 1, :], bsb[:, :])

                ctxf = ctxf_pool.tile([128, L], BF, tag="ctxf", name=f"ctxf{f}")
                cxs = ctxf_pool.tile(
                    [128, 2, 2, 512], BF, tag="cxs", name=f"cxs{f}", bufs=1
                )
                cx_in = dram.tile(
                    [2, 2, 128, 512], BF, tag=f"cxin{f}", name=f"cxin{f}"
                )
                for qh in range(2):
                    cps = [
                        ctx_ps.tile([DH + 1, 512], F32, tag="ctx", name=f"cps{i}")
                        for i in range(2)
                    ]
                    n_kt = 4 * qh + 4  # live k-tiles for this q-half
                    sps_at = {}

                    def emit_scores(kt):
                        sps = [
                            sc_ps.tile([128, 512], F32, tag="sc", name=f"sps{i}")
                            for i in range(2)
                        ]
                        # PE issue is ~216ns/instr regardless of width, so
                        # use one full 512-col matmul pair when both q
                        # halves are live; 256-col only on the causal edge.
                        full = kt <= 4 * qh + 1
                        qsl = (
                            slice(qh * 512, qh * 512 + 512)
                            if full
                            else slice(qh * 512 + 256, qh * 512 + 512)
                        )
                        osl = slice(0, 512) if full else slice(256, 512)
                        ksl = slice(kt * 128, (kt + 1) * 128)
                        for hh in range(2):
                            nc.tensor.matmul(
                                sps[hh][:, osl],
                                kpk[hh][:, :, ksl],
                                qpk[hh][:, :, qsl],
                                start=True,
                                stop=True,
                                perf_mode=mybir.MatmulPerfMode.DoubleRow,
                            )
                        # mask only on diagonal-band halves
                        for qbh in range(2):
                            qb = 2 * qh + qbh
                            if kt in (2 * qb, 2 * qb + 1):
                                msl = slice(qbh * 256, qbh * 256 + 256)
                                for hh in range(2):
                                    nc.vector.tensor_add(
                                        sps[hh][:, msl],
                                        sps[hh][:, msl],
                                        mask_sb[:, kt % 2, :],
                                    )
                        # exp -> bf16 attn tiles (dead qb0 half zeroed so
                        # the full-width ctx matmul accumulates one group
                        # per PSUM bank)
                        at = [
                            attn_pool.tile([128, 512], BF, tag="attn", name=f"at{i}")
                            for i in range(2)
                        ]
                        for hh in range(2):
                            if not full:
                                nc.vector.memset(at[hh][:, 0:256], 0.0)
                            nc.scalar.activation(
                                at[hh][:, osl], sps[hh][:, osl], AF.Exp, scale=SCALE
                            )
                        sps_at[kt] = at

                    def emit_ctx(kt):
                        at = sps_at.pop(kt)
                        for hh in range(2):
                            h = 2 * f + hh
                            nc.tensor.matmul(
                                cps[hh][:, :],
                                vnat[:, kt, h, :],
                                at[hh][:, :],
                                start=(kt == 0),
                                stop=(kt == n_kt - 1),
                            )

                    prev = None
                    for kt in range(n_kt):
                        emit_scores(kt)
                        if prev is not None:
                            emit_ctx(prev)
                        prev = kt
                    emit_ctx(prev)

                    # normalize: divide ctx rows by the attn row-sums that
                    # accumulated in psum row DH (sum staged to SBUF for the
                    # fast custom-DVE reciprocal, which is SBUF-only)
                    for hh in range(2):
                        p0 = hh * 64
                        sr = small.tile([1, 512], F32, tag="sr", bufs=2)
                        nc.scalar.copy(sr[:, :], cps[hh][DH : DH + 1, :])
                        inv = small.tile([1, 512], F32, tag="inv", bufs=2)
                        nc.vector.reciprocal_approx_fast(inv[:, :], sr[:, :])
                        inv_bc = small.tile([64, 512], F32, tag="invbc", bufs=2)
                        nc.gpsimd.partition_broadcast(inv_bc[:, :], inv[:, :])
                        nc.vector.tensor_mul(
                            ctxf[p0 : p0 + 64, qh * 512 : (qh + 1) * 512],
                            cps[hh][0:DH, :],
                            inv_bc[:, :],
                        )

                    # stage this q-half (= dest-rank chunk) for the
                    # exchange as soon as it is normalized
                    for s2 in range(2):
                        nc.vector.tensor_mul(
                            cxs[:, qh, s2, :],
                            ctxf[:, qh * 512 : (qh + 1) * 512],
                            sel_sb[:, s2, :],
                        )
                        nc.sync.dma_start(
                            out=cx_in[qh, s2, :, :], in_=cxs[:, qh, s2, :]
                        )

                # -- exchange fold ctx halves with the pair core.
                # AllToAll isn't available for 2-core groups, so emulate it
                # with a ReduceScatter over [dest d][chan-slot s] staging
                # where slot s != own-half is zeroed via the host-provided
                # 0/1 `sel` tensor (x + 0 is exact in bf16). Rank d then
                # receives [ctx_half0, ctx_half1] for its own rows. --
                cx_out = dram.tile(
                    [2, 128, 512], BF, tag=f"cxout{f}", name=f"cxout{f}"
                )
                nc.gpsimd.collective_compute(
                    "ReduceScatter",
                    ALU.add,
                    replica_groups=GROUPS,
                    ins=[cx_in.opt()],
                    outs=[cx_out.opt()],
                )
                for s2 in range(2):
                    nc.sync.dma_start(
                        out=ctx_all[:, s2 * NQF + f, :], in_=cx_out[s2, :, :]
                    )

            emit_vproj()
            emit_proj(0)
            for f in range(NQF):
                if f + 1 < NQF:
                    emit_proj(f + 1)
                emit_attention(f)

            # ---- out-projection over full 1024 channels for own rows ----
            # 8 chains (lt, dh). Chains for lt 0,1,3 are partially
            # accumulated (folds 0-2 contributions + residual) right after
            # fold-3 attention, filling the PE idle window while fold 3's
            # normalize/exchange runs; the fold-3 contributions and the lt2
            # chains run after the last readback.
            early_chains = [(lt, dh) for lt in (0, 1, 3) for dh in range(2)]
            late_chains = [(2, 0), (2, 1)]
            pools = [sc_ps, proj_ps]
            ptags = ["sc", "proj"]
            ops = {}
            for i, ch in enumerate(early_chains):
                ops[ch] = pools[i % 2].tile(
                    [128, 512], F32, tag=ptags[i % 2], name=f"opse{i}"
                )

            def op_mm(ch, cf, start, stop):
                lt, dh = ch
                nc.tensor.matmul(
                    ops[ch][:, :],
                    ctx_all[:, cf, lt * 128 : (lt + 1) * 128],
                    wout_sb[:, cf, dh * 512 : (dh + 1) * 512],
                    start=start,
                    stop=stop,
                )

            def op_eye(ch, stop):
                lt, dh = ch
                nc.tensor.matmul(
                    ops[ch][:, :],
                    eye_sb[:, :],
                    ev_sb[lt][:, dh * 512 : (dh + 1) * 512],
                    start=False,
                    stop=stop,
                )

            for cf in (0, 1, 2, 4, 5, 6):
                for ch in early_chains:
                    op_mm(ch, cf, start=(cf == 0), stop=False)
            for ch in early_chains:
                op_eye(ch, stop=False)
            # ---- late part: fold-3 contributions ----
            for cf in (3, 7):
                for ch in early_chains:
                    op_mm(ch, cf, start=False, stop=(cf == 7))
            for i, ch in enumerate(late_chains):
                ops[ch] = pools[i % 2].tile(
                    [128, 512], F32, tag=ptags[i % 2], name=f"opsl{i}"
                )
            for cf in range(8):
                for ch in late_chains:
                    op_mm(ch, cf, start=(cf == 0), stop=False)
            for ch in late_chains:
                op_eye(ch, stop=True)

            # ---- evict + LayerNorm per row tile ----
            for lt in (0, 1, 3, 2):
                lsl = slice(lt * 128, (lt + 1) * 128)
                xt_ = ln_pool.tile([128, D], F32, tag="x", name=f"x{lt}")
                nc.scalar.copy(xt_[:, 0:512], ops[(lt, 0)][:, :])
                nc.scalar.copy(xt_[:, 512:1024], ops[(lt, 1)][:, :])
                stats = small.tile([128, 2, 6], F32, tag="stats")
                nc.vector.bn_stats(out=stats[:, 0, :], in_=xt_[:, 0:512])
                nc.vector.bn_stats(out=stats[:, 1, :], in_=xt_[:, 512:1024])
                mv = small.tile([128, 2], F32, tag="mv")
                nc.vector.bn_aggr(out=mv[:, :], in_=stats[:, :, :])
                std = small.tile([128, 1], F32, tag="std")
                nc.scalar.activation(std[:, :], mv[:, 1:2], AF.Sqrt, bias=eps_sb[:, :])
                rstd = small.tile([128, 1], F32, tag="rstd")
                nc.vector.reciprocal(rstd[:, :], std[:, :])
                negmb = small.tile([128, 1], F32, tag="negmb")
                nc.vector.scalar_tensor_tensor(
                    negmb[:, :],
                    mv[:, 0:1],
                    -1.0,
                    rstd[:, :],
                    op0=ALU.mult,
                    op1=ALU.mult,
                )
                dacc = small.tile([128, 1], F32, tag="dacc")
                nc.vector.affine_mul_reduce(
                    xt_[:, :],
                    dacc[:, :],
                    xt_[:, :],
                    gb_bc[:, 0, :],
                    scale=rstd[:, :],
                    bias=negmb[:, :],
                )
                nc.vector.tensor_add(xt_[:, :], xt_[:, :], gb_bc[:, 1, :])
                nc.sync.dma_start(out=out[lsl, :], in_=xt_[:, :])

    nc.compile()
    return nc


def _get_nc():
    if "nc" not in _NC_CACHE:
        _NC_CACHE["nc"] = build_nc()
    return _NC_CACHE["nc"]


def kernel(
    e_v, e_a0, e_a1, Wqv, bqv, Wkv, bkv, Wvv, bvv,
    Wqa0, bqa0, Wka0, bka0, Wqa1, bqa1, Wka1, bka1,
    Wout, bout, ln_gamma, ln_beta, attn_mask,
):
    global LAST_RESULT
    f = np.asarray
    e_v, e_a0, e_a1 = f(e_v), f(e_a0), f(e_a1)
    attn_mask = f(attn_mask)
    c32 = lambda a: np.ascontiguousarray(a, dtype=np.float32)
    cbf = lambda a: np.ascontiguousarray(np.asarray(a, dtype=np.float32).astype(BF16))
    F8NP = ml_dtypes.float8_e4m3
    cf8 = lambda a: np.ascontiguousarray(np.asarray(a, dtype=np.float32).astype(F8NP))

    wq_full = {"v": f(Wqv), "a0": f(Wqa0), "a1": f(Wqa1)}
    wk_full = {"v": f(Wkv), "a0": f(Wka0), "a1": f(Wka1)}
    bq_full = {"v": f(bqv), "a0": f(bqa0), "a1": f(bqa1)}
    bk_full = {"v": f(bkv), "a0": f(bka0), "a1": f(bka1)}

    xts = {}
    xt8s = {}
    maskABs = {}
    ev_all = {"v": e_v, "a0": e_a0, "a1": e_a1}
    for b in range(B):
        xts[b] = {"v": cbf(e_v[b].T)}
        xt8s[b] = {s: cf8(ev_all[s][b].T) for s in STREAMS}
        mT = f(attn_mask[b, 0]).T * (1.0 / SCALE)
        # diagonal-band mask patterns: A = (kt == 2*qb), B = (kt == 2*qb+1)
        maskABs[b] = cbf(np.stack([mT[0:128, 0:256], mT[128:256, 0:256]], axis=1))

    def fold_slice(w, S):
        # [D, C] slice -> [NQF, D, 128] fold-major, fp8 with x64 pre-scale
        # (descaled in the projection eviction) to stay clear of e4m3
        # subnormals
        ws = np.asarray(w[:, S], dtype=np.float32) * 64.0  # [D, C]
        return np.ascontiguousarray(
            ws.reshape(D, NQF, 128).transpose(1, 0, 2).astype(F8NP)
        )

    Wout_f = f(Wout).astype(np.float32)
    bout_f = f(bout).astype(np.float32)
    # [128 p, 8 cf, D] with cf = s*4+f mapping Wout rows s*512+f*128+p
    wout8 = np.ascontiguousarray(
        Wout_f.reshape(2, NQF, 128, D).transpose(2, 0, 1, 3).astype(BF16)
    ).reshape(128, 8, D)

    in_maps = []
    for c in range(NCORES):
        b, hh = c // 2, c % 2
        S = slice(hh * C, (hh + 1) * C)
        m = {}
        m["xt_v"] = xts[b]["v"]
        for s in STREAMS:
            m[f"xt8_{s}"] = xt8s[b][s]
            m[f"wq_{s}"] = fold_slice(wq_full[s], S)
            m[f"wk_{s}"] = fold_slice(wk_full[s], S)
            m[f"bq_{s}"] = c32(bq_full[s][S])
            m[f"bk_{s}"] = c32(bk_full[s][S])
        m["wv"] = cbf(f(Wvv)[:, S])
        m["bv"] = cbf(f(bvv)[S]).reshape(1, C)
        m["wout8"] = wout8
        m["maskAB"] = maskABs[b]
        selv = np.zeros((128, 2, 512), dtype=np.float32)
        selv[:, hh, :] = 1.0
        m["sel"] = cbf(selv)
        m["ev_res"] = cbf(e_v[b, hh * 512 : (hh + 1) * 512, :] + bout_f[None, :])
        m["eye"] = cbf(np.eye(128, dtype=np.float32))
        m["gamma"] = c32(f(ln_gamma)).reshape(1, D)
        m["beta"] = c32(f(ln_beta)).reshape(1, D)
        in_maps.append(m)

    nc = _get_nc()
    trace = bool(os.environ.get("KERNEL_TRACE"))
    res = run_bass_kernel_spmd(
        nc, in_maps, core_ids=list(range(NCORES)), trace=trace
    )
    LAST_RESULT = res

    out_full = np.empty((B, L, D), dtype=np.float32)
    for c in range(NCORES):
        b, hh = c // 2, c % 2
        out_full[b, hh * 512 : (hh + 1) * 512, :] = res.results[c]["out"]
    return out_full
